# revision 20
# baseline (speedup 1.0000x reference)
"""Trainium2 Bass kernel for nn_CLIP_GCN_Model (2-layer GCN + MLP + contrastive loss).

Reformulation (validated numerically):
  out = mean_i(label_i * (lse_i - logits_ii)) + 1.0
(the triplet term of the reference is identically 1.0).

GCN layer: out = S @ (x @ W) + b where S = D^-1/2 (A+I) D^-1/2.
Layer 1 runs over all 10240 (padded) nodes: 80 dst-chunks of 128 nodes,
assigned to (core, slot) with per-slot tile counts T1[s]; per chunk the
distinct source rows are gathered (dedup) and aggregated with a coefficient
matrix C (TensorE matmuls in PSUM), then W_g1 + bias + relu -> h.
h is AllGathered (two halves, Shared-output fast path) into h_t.

Layer 2 only computes the rows actually consumed by the loss: each core owns
batch rows [512c, 512c+512) (4 row-tiles of 128). Per row-tile the distinct
in-edge sources of the rows' label nodes are gathered from h_t (split into
h_t-half-0 / half-1 groups so half-0 aggregation starts right after the first
AllGather) and aggregated directly into [128 rows, 256]; W_g2 is applied via
transposed matmuls producing txtT [512, rows] in SBUF directly.

The image MLP is data-parallel: each core encodes its own 512 images
(transposed layout), AllGathers the result (Shared), and the full [512, 4096]
imgT is used as logits rhs. The diagonal logits are computed locally as
columnwise dots of txtT with the core's own image block. Softmax skips the
row-max subtraction when a host-side bound check shows exp cannot overflow
(max |logit| ~ 10 for this data regime); otherwise a stable variant is built.
"""

import os
import numpy as np
import ml_dtypes

BF16 = ml_dtypes.bfloat16
F8 = ml_dtypes.float8_e4m3   # TRN fp8e4 (max 240)

N_NODES = 10000
NPAD = 10240
D = 512
Hdim = 256
BATCH = 4096
NCORES = 8
P = 128
NCHUNK = NPAD // P          # 80
CPC = NCHUNK // NCORES      # 10 slots per core
NPC = NPAD // NCORES        # 1280 nodes per core
RT = 4                      # row tiles per core (512 rows each core)
NT = BATCH // 512           # 8 column tiles of 512


def _wrap16(idx, n):
    """Layout indices for dma_gather: element i -> [i%16, i//16], replicated to 128 partitions."""
    assert len(idx) == n and n % 16 == 0
    base = idx.astype(np.int16).reshape(n // 16, 16).T  # [16, n/16]
    return np.ascontiguousarray(np.tile(base, (8, 1)))  # [128, n/16]


def _prep(inputs):
    """Host-side layout/sharding prep."""
    x = np.asarray(inputs["x_nodes"], dtype=np.float32)
    image = np.asarray(inputs["image"], dtype=np.float32)
    ei = np.asarray(inputs["edge_index"]).astype(np.int64)
    label = np.asarray(inputs["label"]).astype(np.int64)
    src, dst = ei[0], ei[1]

    deg = np.ones(N_NODES, np.float32)
    np.add.at(deg, dst, 1.0)
    dinv = (1.0 / np.sqrt(deg)).astype(np.float32)

    # in-edges grouped by dst (sorted once)
    order = np.argsort(dst, kind="stable")
    src_s, dst_s = src[order], dst[order]
    bound = np.searchsorted(dst_s, np.arange(N_NODES + 1))

    # ---------------- L1: per-chunk dedup + balanced (core,slot) assignment --
    chunk_src = []      # distinct sources per chunk
    chunk_C = []        # [n_distinct, 128] fp32 coef
    for c in range(NCHUNK):
        n0, n1 = c * P, min((c + 1) * P, N_NODES)
        if n0 >= N_NODES:
            chunk_src.append(np.zeros(1, np.int64))
            chunk_C.append(np.zeros((1, P), np.float32))
            continue
        e0, e1 = bound[n0], bound[n1]
        es, ed = src_s[e0:e1], dst_s[e0:e1]
        selfn = np.arange(n0, n1)
        all_s = np.concatenate([es, selfn])
        all_d = np.concatenate([ed, selfn]) - n0
        coef = np.concatenate([dinv[es] * dinv[ed], dinv[selfn] ** 2])
        uniq, inv = np.unique(all_s, return_inverse=True)
        C = np.zeros((len(uniq), P), np.float32)
        np.add.at(C, (inv, all_d), coef)
        chunk_src.append(uniq)
        chunk_C.append(C)

    counts = np.array([len(s) for s in chunk_src])
    rank = np.argsort(-counts, kind="stable")
    a_k = np.zeros(NCHUNK, np.int64)   # chunk -> core
    s_k = np.zeros(NCHUNK, np.int64)   # chunk -> slot
    T1 = []
    for s in range(CPC):
        grp = rank[s * NCORES:(s + 1) * NCORES]
        a_k[grp] = np.arange(NCORES)
        s_k[grp] = s
        T1.append(int(np.ceil(counts[grp].max() / P)))
    T1 = tuple(T1)
    ST1 = sum(T1)
    off1 = np.concatenate([[0], np.cumsum(T1)])

    # node -> gather-table row: slots 0-4 land in h_t0 (AllGather 1, rows
    # core*640 + slot*128 + p); slots 5-9 land in the packed comb_G (AllGather 2,
    # rows core*1664 + (slot-5)*128 + p), keyed with a +5120 offset.
    kk = np.arange(NPAD) // P
    pp_ = np.arange(NPAD) % P
    hrow = np.where(
        s_k[kk] < CPC // 2,
        a_k[kk] * (NPC // 2) + s_k[kk] * P + pp_,
        NPAD // 2 + a_k[kk] * (NPC // 2) + (s_k[kk] - CPC // 2) * P + pp_,
    )

    gidx1 = np.zeros((NCORES, P, ST1 * 8), np.int16)
    cmat1 = np.zeros((NCORES, P, ST1, P), F8)
    for c in range(NCHUNK):
        cr, sl = a_k[c], s_k[c]
        E_s = T1[sl] * P
        idxs = np.zeros(E_s, np.int64)
        idxs[:counts[c]] = chunk_src[c]
        gidx1[cr, :, off1[sl] * 8:off1[sl + 1] * 8] = _wrap16(idxs, E_s)
        Cp = np.zeros((E_s, P), np.float32)
        Cp[:counts[c]] = chunk_C[c]
        # edge-slot e -> [partition e%128, tile e//128]
        cmat1[cr, :, off1[sl]:off1[sl + 1], :] = \
            Cp.reshape(T1[sl], P, P).transpose(1, 0, 2).astype(F8)

    # ---------------- L2: per-row-tile (labeled dst only), h_t-half split ----
    HALF_N = NPAD // 2
    bins = label.reshape(NCORES, RT, P)   # core c, tile r, row p -> label node
    t2a = np.zeros((NCORES, RT), np.int64)
    t2b = np.zeros((NCORES, RT), np.int64)
    binsrc = {}
    for c in range(NCORES):
        for r in range(RT):
            labs = bins[c, r]
            segs, segd, segc = [], [], []
            for p in range(P):
                v = labs[p]
                e0, e1 = bound[v], bound[v + 1]
                es = src_s[e0:e1]
                segs.append(np.concatenate([es, [v]]))
                segd.append(np.full(len(es) + 1, p, np.int64))
                segc.append(np.concatenate([dinv[es] * dinv[v], [dinv[v] ** 2]]))
            all_s = np.concatenate(segs)
            all_d = np.concatenate(segd)
            coef = np.concatenate(segc)
            hr = hrow[all_s]
            uniq, inv = np.unique(hr, return_inverse=True)
            C = np.zeros((len(uniq), P), np.float32)
            np.add.at(C, (inv, all_d), coef)
            na = int((uniq < HALF_N).sum())   # uniq sorted -> half0 first
            t2a[c, r] = int(np.ceil(max(na, 1) / P))
            t2b[c, r] = int(np.ceil(max(len(uniq) - na, 1) / P))
            binsrc[(c, r)] = (uniq, C, na)
    T2A = tuple(int(t2a[:, r].max()) for r in range(RT))
    T2B = tuple(int(t2b[:, r].max()) for r in range(RT))
    ST2 = sum(T2A) + sum(T2B)
    offA = np.concatenate([[0], np.cumsum(T2A)])
    base_b = offA[-1]
    offB = base_b + np.concatenate([[0], np.cumsum(T2B)])

    gidx2 = np.zeros((NCORES, P, ST2 * 8), np.int16)
    cmat2 = np.zeros((NCORES, P, ST2, P), F8)
    for c in range(NCORES):
        for r in range(RT):
            uniq, C, na = binsrc[(c, r)]
            nb = len(uniq) - na
            Ea, Eb = T2A[r] * P, T2B[r] * P
            ia = np.zeros(Ea, np.int64)
            ia[:na] = uniq[:na]                      # rows into h_t[0:5120]
            ib = np.zeros(Eb, np.int64)
            ib[:nb] = uniq[na:] - HALF_N             # rows into h_t[5120:10240]
            gidx2[c, :, offA[r] * 8:offA[r + 1] * 8] = _wrap16(ia, Ea)
            gidx2[c, :, offB[r] * 8:offB[r + 1] * 8] = _wrap16(ib, Eb)
            Ca = np.zeros((Ea, P), np.float32)
            Ca[:na] = C[:na]
            Cb = np.zeros((Eb, P), np.float32)
            Cb[:nb] = C[na:]
            cmat2[c, :, offA[r]:offA[r + 1], :] = \
                Ca.reshape(T2A[r], P, P).transpose(1, 0, 2).astype(F8)
            cmat2[c, :, offB[r]:offB[r + 1], :] = \
                Cb.reshape(T2B[r], P, P).transpose(1, 0, 2).astype(F8)

    # ---------------- softmax-stability bound (cheap fp32 host forward) ------
    def _agg_all(xw):
        # fast segment sum via reduceat on the dst-sorted edges
        msg = (dinv[src_s] * dinv[dst_s])[:, None] * xw[src_s]
        agg = np.zeros_like(xw)
        nz = np.flatnonzero(np.diff(np.append(-1, dst_s)))
        agg[dst_s[nz]] = np.add.reduceat(msg, nz, axis=0)
        return agg + (dinv * dinv)[:, None] * xw

    h_np = np.maximum(_agg_all(x @ np.asarray(inputs["W_g1"], np.float32))
                      + np.asarray(inputs["b_g1"], np.float32), 0.0)
    g_np = _agg_all(h_np @ np.asarray(inputs["W_g2"], np.float32)) \
        + np.asarray(inputs["b_g2"], np.float32)
    img_np = np.maximum(image @ np.asarray(inputs["W_img1"], np.float32)
                        + np.asarray(inputs["b_img1"], np.float32), 0.0)
    img_np = np.maximum(img_np @ np.asarray(inputs["W_img2"], np.float32)
                        + np.asarray(inputs["b_img2"], np.float32), 0.0)
    bnd_logit = float(np.linalg.norm(g_np[label], axis=1).max()
                      * np.linalg.norm(img_np, axis=1).max())
    stable = bnd_logit > 60.0

    # ---------------- tensors ------------------------------------------------
    xpad = np.zeros((NPAD, D), np.float32)
    xpad[:N_NODES] = x
    xrow = np.ascontiguousarray(xpad).astype(F8)

    def km(w, kt):  # [K, M] -> [128p, kt, M]
        return np.ascontiguousarray(
            w.reshape(kt, P, w.shape[1]).transpose(1, 0, 2)
        ).astype(BF16)

    shared = {
        "xrow": xrow,
        "wg1": km(np.asarray(inputs["W_g1"], np.float32), 4),       # [128, 4, 256]
        "wg2k": np.ascontiguousarray(
            np.asarray(inputs["W_g2"], np.float32).reshape(2, P, 4, P).transpose(1, 0, 2, 3)
        ).astype(BF16),                                             # [128, 2k, 4d, 128]
        "wi1": np.ascontiguousarray(
            np.asarray(inputs["W_img1"], np.float32).reshape(4, P, 2, P).transpose(1, 0, 2, 3)
        ).astype(BF16),
        "wi2": np.ascontiguousarray(
            np.asarray(inputs["W_img2"], np.float32).reshape(2, P, 4, P).transpose(1, 0, 2, 3)
        ).astype(BF16),
        "bg1": np.asarray(inputs["b_g1"], np.float32).astype(BF16).reshape(1, Hdim),
        "bg2": np.asarray(inputs["b_g2"], np.float32).astype(BF16).reshape(1, D),
        "bi1": np.ascontiguousarray(np.asarray(inputs["b_img1"], np.float32).reshape(2, P).T),
        "bi2": np.ascontiguousarray(np.asarray(inputs["b_img2"], np.float32).reshape(4, P).T),
    }

    imageb = image.astype(BF16)
    percore = []
    for c in range(NCORES):
        imt = np.ascontiguousarray(
            imageb[c * 512:(c + 1) * 512].T.reshape(4, P, 512).transpose(1, 0, 2)
        )  # [128 kpart, 4 kblk, 512 own imgs]
        labf = np.ascontiguousarray(
            label[c * 512:(c + 1) * 512].astype(np.float32).reshape(RT, P).T
        )  # [128, RT]
        percore.append({
            "cmat1": np.ascontiguousarray(cmat1[c]),
            "gidx1": np.ascontiguousarray(gidx1[c]),
            "cmat2": np.ascontiguousarray(cmat2[c]),
            "gidx2": np.ascontiguousarray(gidx2[c]),
            "imt": imt, "labf": labf,
        })
    return shared, percore, (T1, T2A, T2B, stable)


def _build(key):
    """Build the SPMD Bass program."""
    T1, T2A, T2B, stable = key
    import concourse.bass as bass  # noqa: F401
    import concourse.tile as tile
    from concourse import bacc, mybir
    from concourse.masks import make_identity

    fp32 = mybir.dt.float32
    bf16 = mybir.dt.bfloat16
    f8 = mybir.dt.float8e4
    i16 = mybir.dt.int16
    AF = mybir.ActivationFunctionType
    DR = mybir.MatmulPerfMode.DoubleRow
    AX = mybir.AxisListType
    ST1 = sum(T1)
    ST2 = sum(T2A) + sum(T2B)
    offA = [0]
    for t in T2A:
        offA.append(offA[-1] + t)
    offB = [offA[-1]]
    for t in T2B:
        offB.append(offB[-1] + t)
    H5 = CPC // 2
    HALF_N = NPAD // 2

    nc = bacc.Bacc("TRN2", target_bir_lowering=False, debug=False,
                   num_devices=NCORES, num_swdge_queues=4)

    t_xrow = nc.dram_tensor("xrow", [NPAD, D], f8, kind="ExternalInput").ap()
    t_wg1 = nc.dram_tensor("wg1", [P, 4, Hdim], bf16, kind="ExternalInput").ap()
    t_wg2k = nc.dram_tensor("wg2k", [P, 2, 4, P], bf16, kind="ExternalInput").ap()
    t_wi1 = nc.dram_tensor("wi1", [P, 4, 2, P], bf16, kind="ExternalInput").ap()
    t_wi2 = nc.dram_tensor("wi2", [P, 2, 4, P], bf16, kind="ExternalInput").ap()
    t_bg1 = nc.dram_tensor("bg1", [1, Hdim], bf16, kind="ExternalInput").ap()
    t_bg2 = nc.dram_tensor("bg2", [1, D], bf16, kind="ExternalInput").ap()
    t_bi1 = nc.dram_tensor("bi1", [P, 2], fp32, kind="ExternalInput").ap()
    t_bi2 = nc.dram_tensor("bi2", [P, 4], fp32, kind="ExternalInput").ap()
    t_cmat1 = nc.dram_tensor("cmat1", [P, ST1, P], f8, kind="ExternalInput").ap()
    t_gidx1 = nc.dram_tensor("gidx1", [P, ST1 * 8], i16, kind="ExternalInput").ap()
    t_cmat2 = nc.dram_tensor("cmat2", [P, ST2, P], f8, kind="ExternalInput").ap()
    t_gidx2 = nc.dram_tensor("gidx2", [P, ST2 * 8], i16, kind="ExternalInput").ap()
    t_imt = nc.dram_tensor("imt", [P, 4, 512], bf16, kind="ExternalInput").ap()
    t_labf = nc.dram_tensor("labf", [P, RT], fp32, kind="ExternalInput").ap()
    t_out = nc.dram_tensor("partial", [1, 1], fp32, kind="ExternalOutput").ap()

    rg = [list(range(NCORES))]

    with tile.TileContext(nc) as tc:
        from contextlib import ExitStack
        with ExitStack() as ctx:
            dram = ctx.enter_context(tc.tile_pool(name="dram", bufs=1, space="DRAM"))
            const = ctx.enter_context(tc.tile_pool(name="const", bufs=1))
            big = ctx.enter_context(tc.tile_pool(name="big", bufs=1))
            work = ctx.enter_context(tc.tile_pool(name="work", bufs=3))
            gbuf = ctx.enter_context(tc.tile_pool(name="gbuf", bufs=3))
            stat = ctx.enter_context(tc.tile_pool(name="stat", bufs=4))

            h_own0 = dram.tile([H5 * P, Hdim], f8)
            h_t0 = dram.tile([HALF_N, Hdim], f8, addr_space="Shared")
            h_own1 = dram.tile([H5 * P, Hdim], f8)
            h_t1 = dram.tile([HALF_N, Hdim], f8, addr_space="Shared")
            # own image encodings, packed [m, colhalf, p] (fp8)
            imgA = dram.tile([1024, Hdim], f8)
            imgG = dram.tile([NCORES * 1024, Hdim], f8, addr_space="Shared")

            # ---- constants in SBUF ----
            wg1_s = const.tile([P, 4, Hdim], bf16)
            nc.sync.dma_start(out=wg1_s[:], in_=t_wg1[:])
            wg2_s = const.tile([P, 2, 4, P], bf16)
            nc.sync.dma_start(out=wg2_s[:], in_=t_wg2k[:])
            wi1_s = const.tile([P, 4, 2, P], bf16)
            nc.sync.dma_start(out=wi1_s[:], in_=t_wi1[:])
            wi2_s = const.tile([P, 2, 4, P], bf16)
            nc.sync.dma_start(out=wi2_s[:], in_=t_wi2[:])
            bg1_s = const.tile([1, Hdim], bf16)
            nc.sync.dma_start(out=bg1_s[:], in_=t_bg1[:])
            bg2_s = const.tile([1, D], bf16)
            nc.sync.dma_start(out=bg2_s[:], in_=t_bg2[:])
            bi1_s = const.tile([P, 2], fp32)
            nc.sync.dma_start(out=bi1_s[:], in_=t_bi1[:])
            bi2_s = const.tile([P, 4], fp32)
            nc.sync.dma_start(out=bi2_s[:], in_=t_bi2[:])
            labf_s = const.tile([P, RT], fp32)
            nc.sync.dma_start(out=labf_s[:], in_=t_labf[:])
            gidx1_s = const.tile([P, ST1 * 8], i16)
            nc.sync.dma_start(out=gidx1_s[:], in_=t_gidx1[:])
            gidx2_s = const.tile([P, ST2 * 8], i16)
            nc.sync.dma_start(out=gidx2_s[:], in_=t_gidx2[:])
            imt_s = const.tile([P, 4, 512], bf16)
            nc.sync.dma_start(out=imt_s[:], in_=t_imt[:])
            ones_row = const.tile([1, P], bf16)
            nc.vector.memset(ones_row[:], 1.0)
            ones_cb = const.tile([P, 1], bf16)
            nc.vector.memset(ones_cb[:], 1.0)
            ones_col = const.tile([P, 1], fp32)
            nc.vector.memset(ones_col[:], 1.0)
            ident_b = const.tile([P, P], bf16)
            make_identity(nc, ident_b[:])

            imgown8 = big.tile([P, 4, 512], f8)     # own images encoded (fp8)
            imgownT = big.tile([P, 4, 512], bf16)   # bf16 copy of the same values
            imgT_s = big.tile([P, 4, BATCH], f8)    # full imgT after AllGather
            txtT8 = big.tile([P, RT, 4, P], f8)     # txtT per row tile (fp8)
            txtT_s = big.tile([P, RT, 4, P], bf16)  # bf16 copy of same values (diag)
            diag_s = stat.tile([P, RT], fp32)
            contrib = stat.tile([P, RT], fp32)

            # ===== image MLP on own 512 images (fills L1 warmup) ==============
            with tc.tile_pool(name="ps_mlp", bufs=2, space="PSUM") as ps_mlp:
                h1t = big.tile([P, 2, 512], bf16)
                for m in range(2):
                    pm = ps_mlp.tile([P, 512], fp32, tag="mlp1")
                    for k in range(4):
                        nc.tensor.matmul(
                            out=pm[:], lhsT=wi1_s[:, k, m, :], rhs=imt_s[:, k, :],
                            start=(k == 0), stop=(k == 3),
                        )
                    nc.scalar.activation(
                        out=h1t[:, m, :], in_=pm[:], func=AF.Relu,
                        bias=bi1_s[:, m:m + 1], scale=1.0,
                    )
                for m in range(4):
                    pm2 = ps_mlp.tile([P, 512], fp32, tag="mlp2")
                    for k in range(2):
                        nc.tensor.matmul(
                            out=pm2[:], lhsT=wi2_s[:, k, m, :], rhs=h1t[:, k, :],
                            start=(k == 0), stop=(k == 1),
                        )
                    nc.scalar.activation(
                        out=imgown8[:, m, :], in_=pm2[:], func=AF.Relu,
                        bias=bi2_s[:, m:m + 1], scale=1.0,
                    )
                    for hcol in range(2):
                        nc.sync.dma_start(
                            out=imgA[m * 256 + hcol * P:m * 256 + (hcol + 1) * P, :],
                            in_=imgown8[:, m, hcol * 256:(hcol + 1) * 256],
                        )
                nc.vector.tensor_copy(out=imgownT[:], in_=imgown8[:])
            # image AllGather: input ready immediately, result needed only by
            # the logits phase -- rides the collective engine early, off the
            # critical path.
            nc.gpsimd.collective_compute(
                "AllGather", mybir.AluOpType.bypass, replica_groups=rg,
                ins=[imgA[:, :]], outs=[imgG[:, :]],
            )


            # ===== GCN layer 1: my 10 slots ===================================
            T1h = (max(T1) + 3) // 4
            o1 = [0]
            for t in T1:
                o1.append(o1[-1] + t)

            def l1_slot(s, ps_ag):
                Ts = T1[s]
                j0 = o1[s]
                cm = gbuf.tile([P, max(T1), P], f8, tag="cm", name="cm")
                nc.scalar.dma_start(out=cm[:, :Ts, :], in_=t_cmat1[:, j0:j0 + Ts, :])
                pa = ps_ag.tile([P, D], fp32, tag="agg1", name="pa")
                qs = [(i * Ts) // 4 for i in range(5)]
                parts = [(qs[i], qs[i + 1]) for i in range(4) if qs[i + 1] > qs[i]]
                ghs = []
                for qi, (a, b) in enumerate(parts):
                    gh = gbuf.tile([P, T1h, D], f8, tag=f"g1_{qi}", name="gh")
                    nc.gpsimd.dma_gather(
                        out_ap=gh[:, :b - a, :], in_ap=t_xrow[:, :],
                        idxs_ap=gidx1_s[:, (j0 + a) * 8:(j0 + b) * 8],
                        num_idxs=(b - a) * P, num_idxs_reg=(b - a) * P,
                        elem_size=D, single_packet=False,
                        queue_num=(s + qi) % 4,
                    )
                    ghs.append(gh)
                for qi, (a, b) in enumerate(parts):
                    j = a
                    while j < b:
                        if j + 1 < b:
                            nc.tensor.matmul(
                                out=pa[:], lhsT=cm[:, j:j + 2, :],
                                rhs=ghs[qi][:, j - a:j - a + 2, :],
                                start=(j == 0), stop=(j + 2 == Ts), perf_mode=DR,
                            )
                            j += 2
                        else:
                            nc.tensor.matmul(
                                out=pa[:], lhsT=cm[:, j, :], rhs=ghs[qi][:, j - a, :],
                                start=(j == 0), stop=(j + 1 == Ts),
                            )
                            j += 1
                a1 = work.tile([P, D], bf16, tag="a1", name="a1")
                nc.vector.tensor_copy(out=a1[:], in_=pa[:])
                a1t = work.tile([P, 4, P], bf16, tag="a1t", name="a1t")
                for k in range(4):
                    pt1 = ps_ag.tile([P, P], bf16, tag="tps1", name="pt1")
                    nc.tensor.transpose(
                        out=pt1[:], in_=a1[:, k * P:(k + 1) * P], identity=ident_b[:]
                    )
                    nc.vector.tensor_copy(out=a1t[:, k, :], in_=pt1[:])
                ph = ps_ag.tile([P, Hdim], fp32, tag="hps", name="ph")
                for k in range(4):
                    nc.tensor.matmul(
                        out=ph[:], lhsT=a1t[:, k, :], rhs=wg1_s[:, k, :],
                        start=(k == 0), stop=False,
                    )
                nc.tensor.matmul(
                    out=ph[:], lhsT=ones_row[:], rhs=bg1_s[:],
                    start=False, stop=True, skip_group_check=True,
                )
                h_sb = work.tile([P, Hdim], f8, tag="h_sb", name="h_sb")
                nc.scalar.activation(out=h_sb[:], in_=ph[:], func=AF.Relu)
                if s < H5:
                    nc.sync.dma_start(out=h_own0[s * P:(s + 1) * P, :], in_=h_sb[:])
                else:
                    nc.sync.dma_start(
                        out=h_own1[(s - H5) * P:(s - H5 + 1) * P, :], in_=h_sb[:]
                    )

            with tc.tile_pool(name="ps_ag", bufs=2, space="PSUM") as ps_ag:
                for s in range(H5):
                    l1_slot(s, ps_ag)
                nc.gpsimd.collective_compute(
                    "AllGather", mybir.AluOpType.bypass, replica_groups=rg,
                    ins=[h_own0[:, :]], outs=[h_t0[:, :]],
                )
                for s in range(H5, CPC):
                    l1_slot(s, ps_ag)

            # full imgT into SBUF (sync queue is drained of h stores by now)
            for n in range(NT):
                for k in range(4):
                    for hcol in range(2):
                        nc.sync.dma_start(
                            out=imgT_s[:, k, n * 512 + hcol * 256:
                                       n * 512 + (hcol + 1) * 256],
                            in_=imgG[n * 1024 + k * 256 + hcol * P:
                                     n * 1024 + k * 256 + (hcol + 1) * P, :],
                        )

            # ===== GCN layer 2, phase A (h_t0 sources; overlaps 2nd AllGather)
            T2m = max(max(T2A), max(T2B))
            a2A_s = big.tile([P, RT, Hdim], bf16)   # partial aggregates (A half)
            c2 = ExitStack()
            ps_l2 = c2.enter_context(tc.tile_pool(name="ps_l2", bufs=1, space="PSUM"))
            ps_tx = ps_l2
            cm2 = big.tile([P, ST2, P], f8)
            nc.scalar.dma_start(out=cm2[:], in_=t_cmat2[:])
            ga_t = []
            for r in range(RT):
                ga = gbuf.tile([P, T2m, Hdim], f8, tag=f"g2a{r}", name="ga", bufs=1)
                nc.gpsimd.dma_gather(
                    out_ap=ga[:, :T2A[r], :], in_ap=h_t0[:, :],
                    idxs_ap=gidx2_s[:, offA[r] * 8:offA[r + 1] * 8],
                    num_idxs=T2A[r] * P, num_idxs_reg=T2A[r] * P,
                    elem_size=Hdim, single_packet=False,
                    queue_num=(2 * r) % 4,
                )
                ga_t.append(ga)
            for r in range(RT):
                pa2 = ps_l2.tile([P, Hdim], fp32, tag="agg2", name="pa2", bufs=2)
                ga = ga_t[r]
                j = 0
                while j < T2A[r]:
                    if j + 1 < T2A[r]:
                        nc.tensor.matmul(
                            out=pa2[:], lhsT=cm2[:, offA[r] + j:offA[r] + j + 2, :],
                            rhs=ga[:, j:j + 2, :], start=(j == 0),
                            stop=(j + 2 == T2A[r]), perf_mode=DR,
                        )
                        j += 2
                    else:
                        nc.tensor.matmul(
                            out=pa2[:], lhsT=cm2[:, offA[r] + j, :], rhs=ga[:, j, :],
                            start=(j == 0), stop=(j + 1 == T2A[r]),
                        )
                        j += 1
                nc.vector.tensor_copy(out=a2A_s[:, r, :], in_=pa2[:])

            # 2nd AllGather: h slots 5-9
            nc.gpsimd.collective_compute(
                "AllGather", mybir.AluOpType.bypass, replica_groups=rg,
                ins=[h_own1[:, :]], outs=[h_t1[:, :]],
            )

            # ===== phase B (comb_G sources) + txtT + diag =====================
            gb_t = []
            for r in range(RT):
                gb = gbuf.tile([P, T2m, Hdim], f8, tag=f"g2b{r}", name="gb", bufs=1)
                nc.gpsimd.dma_gather(
                    out_ap=gb[:, :T2B[r], :], in_ap=h_t1[:, :],
                    idxs_ap=gidx2_s[:, offB[r] * 8:offB[r + 1] * 8],
                    num_idxs=T2B[r] * P, num_idxs_reg=T2B[r] * P,
                    elem_size=Hdim, single_packet=False,
                    queue_num=(2 * r + 1) % 4,
                )
                gb_t.append(gb)
            for r in range(RT):
                pa2 = ps_l2.tile([P, Hdim], fp32, tag="agg2", name="pa2b", bufs=2)
                gb = gb_t[r]
                j = 0
                while j < T2B[r]:
                    if j + 1 < T2B[r]:
                        nc.tensor.matmul(
                            out=pa2[:], lhsT=cm2[:, offB[r] + j:offB[r] + j + 2, :],
                            rhs=gb[:, j:j + 2, :], start=(j == 0),
                            stop=(j + 2 == T2B[r]), perf_mode=DR,
                        )
                        j += 2
                    else:
                        nc.tensor.matmul(
                            out=pa2[:], lhsT=cm2[:, offB[r] + j, :], rhs=gb[:, j, :],
                            start=(j == 0), stop=(j + 1 == T2B[r]),
                        )
                        j += 1
                a2b = work.tile([P, Hdim], bf16, tag="a2b")
                nc.vector.tensor_copy(out=a2b[:], in_=pa2[:])
                a2 = work.tile([P, Hdim], bf16, tag="a2")
                nc.vector.tensor_add(out=a2[:], in0=a2A_s[:, r, :], in1=a2b[:])
                a2t = work.tile([P, 2, P], bf16, tag="a2t")
                for k in range(2):
                    pt = ps_l2.tile([P, P], bf16, tag="tps")
                    nc.tensor.transpose(
                        out=pt[:], in_=a2[:, k * P:(k + 1) * P], identity=ident_b[:]
                    )
                    nc.vector.tensor_copy(out=a2t[:, k, :], in_=pt[:])
                # txtT[d block] = W2[:,d]^T @ agg2^T + b2[d]
                dprod = work.tile([P, 4, P], bf16, tag="dprod")
                for d in range(4):
                    ptx = ps_tx.tile([P, P], fp32, tag="ptx")
                    for k in range(2):
                        nc.tensor.matmul(
                            out=ptx[:], lhsT=wg2_s[:, k, d, :], rhs=a2t[:, k, :],
                            start=(k == 0), stop=False,
                        )
                    nc.tensor.matmul(
                        out=ptx[:], lhsT=bg2_s[:, d * P:(d + 1) * P], rhs=ones_row[:],
                        start=False, stop=True, skip_group_check=True,
                    )
                    nc.vector.tensor_copy(out=txtT8[:, r, d, :], in_=ptx[:])
                    nc.vector.tensor_copy(out=txtT_s[:, r, d, :], in_=txtT8[:, r, d, :])
                    nc.vector.tensor_tensor(
                        out=dprod[:, d, :], in0=txtT_s[:, r, d, :],
                        in1=imgownT[:, d, r * P:(r + 1) * P],
                        op=mybir.AluOpType.mult,
                    )
                pd = ps_tx.tile([P, 1], fp32, tag="pd")
                for d in range(4):
                    nc.tensor.matmul(
                        out=pd[:], lhsT=dprod[:, d, :], rhs=ones_cb[:],
                        start=(d == 0), stop=(d == 3),
                    )
                nc.vector.tensor_copy(out=diag_s[:, r:r + 1], in_=pd[:])
                # ---- logits + row losses for this row tile ----
                sums = stat.tile([P, NT], fp32, tag="sums")
                if stable:
                    banks = []
                for n in range(NT):
                    pl = ps_l2.tile([P, 512], fp32, tag="lg", bufs=2)
                    for g in range(2):
                        nc.tensor.matmul(
                            out=pl[:], lhsT=txtT8[:, r, 2 * g:2 * g + 2, :],
                            rhs=imgT_s[:, 2 * g:2 * g + 2, n * 512:(n + 1) * 512],
                            start=(g == 0), stop=(g == 1), perf_mode=DR,
                        )
                    if stable:
                        banks.append(pl)
                    else:
                        esc = work.tile([P, 512], bf16, tag="esc")
                        nc.scalar.activation(
                            out=esc[:], in_=pl[:], func=AF.Exp,
                            accum_out=sums[:, n:n + 1],
                        )
                if stable:
                    maxes = stat.tile([P, NT], fp32, tag="maxes")
                    for n in range(NT):
                        nc.vector.reduce_max(out=maxes[:, n:n + 1], in_=banks[n][:], axis=AX.X)
                    rmax = stat.tile([P, 1], fp32, tag="rmax")
                    nc.vector.reduce_max(out=rmax[:], in_=maxes[:], axis=AX.X)
                    nrmax = stat.tile([P, 1], fp32, tag="nrmax")
                    nc.scalar.mul(nrmax[:], rmax[:], -1.0)
                    for n in range(NT):
                        esc = work.tile([P, 512], bf16, tag="esc")
                        nc.scalar.activation(
                            out=esc[:], in_=banks[n][:], func=AF.Exp,
                            bias=nrmax[:], scale=1.0, accum_out=sums[:, n:n + 1],
                        )
                ssum = stat.tile([P, 1], fp32, tag="ssum")
                nc.vector.reduce_sum(out=ssum[:], in_=sums[:], axis=AX.X)
                lns = stat.tile([P, 1], fp32, tag="lns")
                nc.scalar.activation(out=lns[:], in_=ssum[:], func=AF.Ln)
                t1 = stat.tile([P, 1], fp32, tag="t1")
                if stable:
                    nc.vector.tensor_add(out=t1[:], in0=rmax[:], in1=lns[:])
                    nc.vector.tensor_sub(out=t1[:], in0=t1[:], in1=diag_s[:, r:r + 1])
                else:
                    nc.vector.tensor_sub(out=t1[:], in0=lns[:], in1=diag_s[:, r:r + 1])
                nc.vector.tensor_mul(
                    out=contrib[:, r:r + 1], in0=t1[:], in1=labf_s[:, r:r + 1]
                )
            c2.close()
            rsum = stat.tile([P, 1], fp32, tag="rsum")
            nc.vector.reduce_sum(out=rsum[:], in_=contrib[:], axis=AX.X)
            with tc.tile_pool(name="ps_fin", bufs=1, space="PSUM") as ps_fin:
                pf = ps_fin.tile([1, 1], fp32)
                nc.tensor.matmul(out=pf[:], lhsT=rsum[:], rhs=ones_col[:], start=True, stop=True)
                fin = stat.tile([1, 1], fp32, tag="fin")
                nc.vector.tensor_copy(out=fin[:], in_=pf[:])
            nc.sync.dma_start(out=t_out[:], in_=fin[:])

    nc.compile()
    return nc


_CACHE = {}


def kernel(**inputs) -> np.ndarray:
    from concourse.bass_utils import run_bass_kernel_spmd

    shared, percore, key = _prep(inputs)
    ckey = (key[0], key[1], key[2], key[3])
    if ckey not in _CACHE:
        _CACHE[ckey] = _build(ckey)
    nc = _CACHE[ckey]

    in_maps = []
    for c in range(NCORES):
        m = dict(shared)
        pc = percore[c]
        m.update({"cmat1": pc["cmat1"], "gidx1": pc["gidx1"],
                  "cmat2": pc["cmat2"], "gidx2": pc["gidx2"],
                  "imt": pc["imt"], "labf": pc["labf"]})
        in_maps.append(m)

    trace = bool(int(os.environ.get("KERNEL_TRACE", "0")))
    try:
        res = run_bass_kernel_spmd(nc, in_maps, core_ids=list(range(NCORES)), trace=trace)
    except Exception:
        # transient NRT/device hiccups have been observed to clear on retry
        res = run_bass_kernel_spmd(nc, in_maps, core_ids=list(range(NCORES)), trace=trace)
    kernel.last_results = res
    total = sum(float(r["partial"][0, 0]) for r in res.results)
    return np.float32(total / BATCH + 1.0)


# revision 23
# speedup vs baseline: 1.1121x; 1.1121x over previous
"""Trainium2 Bass kernel for nn_CLIP_GCN_Model (2-layer GCN + MLP + contrastive loss).

Reformulation (validated numerically):
  out = mean_i(label_i * (lse_i - logits_ii)) + 1.0
(the triplet term of the reference is identically 1.0).

GCN layer: out = S @ (x @ W) + b where S = D^-1/2 (A+I) D^-1/2.
Layer 1 runs over all 10240 (padded) nodes: 80 dst-chunks of 128 nodes,
assigned to (core, slot) with per-slot tile counts T1[s]; per chunk the
distinct source rows are gathered (dedup) and aggregated with a coefficient
matrix C (TensorE matmuls in PSUM), then W_g1 + bias + relu -> h.
h is AllGathered (two halves, Shared-output fast path) into h_t.

Layer 2 only computes the rows actually consumed by the loss: each core owns
batch rows [512c, 512c+512) (4 row-tiles of 128). Per row-tile the distinct
in-edge sources of the rows' label nodes are gathered from h_t (split into
h_t-half-0 / half-1 groups so half-0 aggregation starts right after the first
AllGather) and aggregated directly into [128 rows, 256]; W_g2 is applied via
transposed matmuls producing txtT [512, rows] in SBUF directly.

The image MLP is data-parallel: each core encodes its own 512 images
(transposed layout), AllGathers the result (Shared), and the full [512, 4096]
imgT is used as logits rhs. The diagonal logits are computed locally as
columnwise dots of txtT with the core's own image block. Softmax skips the
row-max subtraction when a host-side bound check shows exp cannot overflow
(max |logit| ~ 10 for this data regime); otherwise a stable variant is built.
"""

import os
import numpy as np
import ml_dtypes

BF16 = ml_dtypes.bfloat16
F8 = ml_dtypes.float8_e4m3   # TRN fp8e4 (max 240)

N_NODES = 10000
NPAD = 10240
D = 512
Hdim = 256
BATCH = 4096
NCORES = 8
P = 128
NCHUNK = NPAD // P          # 80
CPC = NCHUNK // NCORES      # 10 slots per core
NPC = NPAD // NCORES        # 1280 nodes per core
RT = 4                      # row tiles per core (512 rows each core)
NT = BATCH // 512           # 8 column tiles of 512


def _wrap16(idx, n):
    """Layout indices for dma_gather: element i -> [i%16, i//16], replicated to 128 partitions."""
    assert len(idx) == n and n % 16 == 0
    base = idx.astype(np.int16).reshape(n // 16, 16).T  # [16, n/16]
    return np.ascontiguousarray(np.tile(base, (8, 1)))  # [128, n/16]


def _prep(inputs):
    """Host-side layout/sharding prep."""
    x = np.asarray(inputs["x_nodes"], dtype=np.float32)
    image = np.asarray(inputs["image"], dtype=np.float32)
    ei = np.asarray(inputs["edge_index"]).astype(np.int64)
    label = np.asarray(inputs["label"]).astype(np.int64)
    src, dst = ei[0], ei[1]

    deg = np.ones(N_NODES, np.float32)
    np.add.at(deg, dst, 1.0)
    dinv = (1.0 / np.sqrt(deg)).astype(np.float32)

    # in-edges grouped by dst (sorted once)
    order = np.argsort(dst, kind="stable")
    src_s, dst_s = src[order], dst[order]
    bound = np.searchsorted(dst_s, np.arange(N_NODES + 1))

    # ---------------- L1: per-chunk dedup + balanced (core,slot) assignment --
    chunk_src = []      # distinct sources per chunk
    chunk_C = []        # [n_distinct, 128] fp32 coef
    for c in range(NCHUNK):
        n0, n1 = c * P, min((c + 1) * P, N_NODES)
        if n0 >= N_NODES:
            chunk_src.append(np.zeros(1, np.int64))
            chunk_C.append(np.zeros((1, P), np.float32))
            continue
        e0, e1 = bound[n0], bound[n1]
        es, ed = src_s[e0:e1], dst_s[e0:e1]
        selfn = np.arange(n0, n1)
        all_s = np.concatenate([es, selfn])
        all_d = np.concatenate([ed, selfn]) - n0
        coef = np.concatenate([dinv[es] * dinv[ed], dinv[selfn] ** 2])
        uniq, inv = np.unique(all_s, return_inverse=True)
        C = np.zeros((len(uniq), P), np.float32)
        np.add.at(C, (inv, all_d), coef)
        chunk_src.append(uniq)
        chunk_C.append(C)

    counts = np.array([len(s) for s in chunk_src])
    rank = np.argsort(-counts, kind="stable")
    a_k = np.zeros(NCHUNK, np.int64)   # chunk -> core
    s_k = np.zeros(NCHUNK, np.int64)   # chunk -> slot
    T1 = []
    for s in range(CPC):
        grp = rank[s * NCORES:(s + 1) * NCORES]
        a_k[grp] = np.arange(NCORES)
        s_k[grp] = s
        T1.append(int(np.ceil(counts[grp].max() / P)))
    T1 = tuple(T1)
    ST1 = sum(T1)
    off1 = np.concatenate([[0], np.cumsum(T1)])

    # node -> gather-table row: slots 0-4 land in h_t0 (AllGather 1, rows
    # core*640 + slot*128 + p); slots 5-9 land in the packed comb_G (AllGather 2,
    # rows core*1664 + (slot-5)*128 + p), keyed with a +5120 offset.
    kk = np.arange(NPAD) // P
    pp_ = np.arange(NPAD) % P
    hrow = np.where(
        s_k[kk] < CPC // 2,
        a_k[kk] * (NPC // 2) + s_k[kk] * P + pp_,
        NPAD // 2 + a_k[kk] * 1664 + (s_k[kk] - CPC // 2) * P + pp_,
    )

    gidx1 = np.zeros((NCORES, P, ST1 * 8), np.int16)
    cmat1 = np.zeros((NCORES, P, ST1, P), F8)
    for c in range(NCHUNK):
        cr, sl = a_k[c], s_k[c]
        E_s = T1[sl] * P
        idxs = np.zeros(E_s, np.int64)
        idxs[:counts[c]] = chunk_src[c]
        gidx1[cr, :, off1[sl] * 8:off1[sl + 1] * 8] = _wrap16(idxs, E_s)
        Cp = np.zeros((E_s, P), np.float32)
        Cp[:counts[c]] = chunk_C[c]
        # edge-slot e -> [partition e%128, tile e//128]
        cmat1[cr, :, off1[sl]:off1[sl + 1], :] = \
            Cp.reshape(T1[sl], P, P).transpose(1, 0, 2).astype(F8)

    # ---------------- L2: per-row-tile (labeled dst only), h_t-half split ----
    HALF_N = NPAD // 2
    bins = label.reshape(NCORES, RT, P)   # core c, tile r, row p -> label node
    t2a = np.zeros((NCORES, RT), np.int64)
    t2b = np.zeros((NCORES, RT), np.int64)
    binsrc = {}
    for c in range(NCORES):
        for r in range(RT):
            labs = bins[c, r]
            segs, segd, segc = [], [], []
            for p in range(P):
                v = labs[p]
                e0, e1 = bound[v], bound[v + 1]
                es = src_s[e0:e1]
                segs.append(np.concatenate([es, [v]]))
                segd.append(np.full(len(es) + 1, p, np.int64))
                segc.append(np.concatenate([dinv[es] * dinv[v], [dinv[v] ** 2]]))
            all_s = np.concatenate(segs)
            all_d = np.concatenate(segd)
            coef = np.concatenate(segc)
            hr = hrow[all_s]
            uniq, inv = np.unique(hr, return_inverse=True)
            C = np.zeros((len(uniq), P), np.float32)
            np.add.at(C, (inv, all_d), coef)
            na = int((uniq < HALF_N).sum())   # uniq sorted -> half0 first
            t2a[c, r] = int(np.ceil(max(na, 1) / P))
            t2b[c, r] = int(np.ceil(max(len(uniq) - na, 1) / P))
            binsrc[(c, r)] = (uniq, C, na)
    T2A = tuple(int(t2a[:, r].max()) for r in range(RT))
    T2B = tuple(int(t2b[:, r].max()) for r in range(RT))
    ST2 = sum(T2A) + sum(T2B)
    offA = np.concatenate([[0], np.cumsum(T2A)])
    base_b = offA[-1]
    offB = base_b + np.concatenate([[0], np.cumsum(T2B)])

    gidx2 = np.zeros((NCORES, P, ST2 * 8), np.int16)
    cmat2 = np.zeros((NCORES, P, ST2, P), F8)
    for c in range(NCORES):
        for r in range(RT):
            uniq, C, na = binsrc[(c, r)]
            nb = len(uniq) - na
            Ea, Eb = T2A[r] * P, T2B[r] * P
            ia = np.zeros(Ea, np.int64)
            ia[:na] = uniq[:na]                      # rows into h_t[0:5120]
            ib = np.zeros(Eb, np.int64)
            ib[:nb] = uniq[na:] - HALF_N             # rows into h_t[5120:10240]
            gidx2[c, :, offA[r] * 8:offA[r + 1] * 8] = _wrap16(ia, Ea)
            gidx2[c, :, offB[r] * 8:offB[r + 1] * 8] = _wrap16(ib, Eb)
            Ca = np.zeros((Ea, P), np.float32)
            Ca[:na] = C[:na]
            Cb = np.zeros((Eb, P), np.float32)
            Cb[:nb] = C[na:]
            cmat2[c, :, offA[r]:offA[r + 1], :] = \
                Ca.reshape(T2A[r], P, P).transpose(1, 0, 2).astype(F8)
            cmat2[c, :, offB[r]:offB[r + 1], :] = \
                Cb.reshape(T2B[r], P, P).transpose(1, 0, 2).astype(F8)

    # ---------------- softmax-stability bound (cheap fp32 host forward) ------
    def _agg_all(xw):
        # fast segment sum via reduceat on the dst-sorted edges
        msg = (dinv[src_s] * dinv[dst_s])[:, None] * xw[src_s]
        agg = np.zeros_like(xw)
        nz = np.flatnonzero(np.diff(np.append(-1, dst_s)))
        agg[dst_s[nz]] = np.add.reduceat(msg, nz, axis=0)
        return agg + (dinv * dinv)[:, None] * xw

    h_np = np.maximum(_agg_all(x @ np.asarray(inputs["W_g1"], np.float32))
                      + np.asarray(inputs["b_g1"], np.float32), 0.0)
    g_np = _agg_all(h_np @ np.asarray(inputs["W_g2"], np.float32)) \
        + np.asarray(inputs["b_g2"], np.float32)
    img_np = np.maximum(image @ np.asarray(inputs["W_img1"], np.float32)
                        + np.asarray(inputs["b_img1"], np.float32), 0.0)
    img_np = np.maximum(img_np @ np.asarray(inputs["W_img2"], np.float32)
                        + np.asarray(inputs["b_img2"], np.float32), 0.0)
    bnd_logit = float(np.linalg.norm(g_np[label], axis=1).max()
                      * np.linalg.norm(img_np, axis=1).max())
    stable = bnd_logit > 60.0

    # ---------------- tensors ------------------------------------------------
    xpad = np.zeros((NPAD, D), np.float32)
    xpad[:N_NODES] = x
    xrow = np.ascontiguousarray(xpad).astype(F8)

    def km(w, kt):  # [K, M] -> [128p, kt, M]
        return np.ascontiguousarray(
            w.reshape(kt, P, w.shape[1]).transpose(1, 0, 2)
        ).astype(BF16)

    shared = {
        "xrow": xrow,
        "wg1": km(np.asarray(inputs["W_g1"], np.float32), 4),       # [128, 4, 256]
        "wg2k": np.ascontiguousarray(
            np.asarray(inputs["W_g2"], np.float32).reshape(2, P, 4, P).transpose(1, 0, 2, 3)
        ).astype(BF16),                                             # [128, 2k, 4d, 128]
        "wi1": np.ascontiguousarray(
            np.asarray(inputs["W_img1"], np.float32).reshape(4, P, 2, P).transpose(1, 0, 2, 3)
        ).astype(BF16),
        "wi2": np.ascontiguousarray(
            np.asarray(inputs["W_img2"], np.float32).reshape(2, P, 4, P).transpose(1, 0, 2, 3)
        ).astype(BF16),
        "bg1": np.asarray(inputs["b_g1"], np.float32).astype(BF16).reshape(1, Hdim),
        "bg2": np.asarray(inputs["b_g2"], np.float32).astype(BF16).reshape(1, D),
        "bi1": np.ascontiguousarray(np.asarray(inputs["b_img1"], np.float32).reshape(2, P).T),
        "bi2": np.ascontiguousarray(np.asarray(inputs["b_img2"], np.float32).reshape(4, P).T),
    }

    imageb = image.astype(BF16)
    percore = []
    for c in range(NCORES):
        imt = np.ascontiguousarray(
            imageb[c * 512:(c + 1) * 512].T.reshape(4, P, 512).transpose(1, 0, 2)
        )  # [128 kpart, 4 kblk, 512 own imgs]
        labf = np.ascontiguousarray(
            label[c * 512:(c + 1) * 512].astype(np.float32).reshape(RT, P).T
        )  # [128, RT]
        percore.append({
            "cmat1": np.ascontiguousarray(cmat1[c]),
            "gidx1": np.ascontiguousarray(gidx1[c]),
            "cmat2": np.ascontiguousarray(cmat2[c]),
            "gidx2": np.ascontiguousarray(gidx2[c]),
            "imt": imt, "labf": labf,
        })
    return shared, percore, (T1, T2A, T2B, stable)


def _build(key):
    """Build the SPMD Bass program."""
    T1, T2A, T2B, stable = key
    import concourse.bass as bass  # noqa: F401
    import concourse.tile as tile
    from concourse import bacc, mybir
    from concourse.masks import make_identity

    fp32 = mybir.dt.float32
    bf16 = mybir.dt.bfloat16
    f8 = mybir.dt.float8e4
    i16 = mybir.dt.int16
    AF = mybir.ActivationFunctionType
    DR = mybir.MatmulPerfMode.DoubleRow
    AX = mybir.AxisListType
    ST1 = sum(T1)
    ST2 = sum(T2A) + sum(T2B)
    offA = [0]
    for t in T2A:
        offA.append(offA[-1] + t)
    offB = [offA[-1]]
    for t in T2B:
        offB.append(offB[-1] + t)
    H5 = CPC // 2
    HALF_N = NPAD // 2

    nc = bacc.Bacc("TRN2", target_bir_lowering=False, debug=False,
                   num_devices=NCORES, num_swdge_queues=4)

    t_xrow = nc.dram_tensor("xrow", [NPAD, D], f8, kind="ExternalInput").ap()
    t_wg1 = nc.dram_tensor("wg1", [P, 4, Hdim], bf16, kind="ExternalInput").ap()
    t_wg2k = nc.dram_tensor("wg2k", [P, 2, 4, P], bf16, kind="ExternalInput").ap()
    t_wi1 = nc.dram_tensor("wi1", [P, 4, 2, P], bf16, kind="ExternalInput").ap()
    t_wi2 = nc.dram_tensor("wi2", [P, 2, 4, P], bf16, kind="ExternalInput").ap()
    t_bg1 = nc.dram_tensor("bg1", [1, Hdim], bf16, kind="ExternalInput").ap()
    t_bg2 = nc.dram_tensor("bg2", [1, D], bf16, kind="ExternalInput").ap()
    t_bi1 = nc.dram_tensor("bi1", [P, 2], fp32, kind="ExternalInput").ap()
    t_bi2 = nc.dram_tensor("bi2", [P, 4], fp32, kind="ExternalInput").ap()
    t_cmat1 = nc.dram_tensor("cmat1", [P, ST1, P], f8, kind="ExternalInput").ap()
    t_gidx1 = nc.dram_tensor("gidx1", [P, ST1 * 8], i16, kind="ExternalInput").ap()
    t_cmat2 = nc.dram_tensor("cmat2", [P, ST2, P], f8, kind="ExternalInput").ap()
    t_gidx2 = nc.dram_tensor("gidx2", [P, ST2 * 8], i16, kind="ExternalInput").ap()
    t_imt = nc.dram_tensor("imt", [P, 4, 512], bf16, kind="ExternalInput").ap()
    t_labf = nc.dram_tensor("labf", [P, RT], fp32, kind="ExternalInput").ap()
    t_out = nc.dram_tensor("partial", [1, 1], fp32, kind="ExternalOutput").ap()

    rg = [list(range(NCORES))]

    with tile.TileContext(nc) as tc:
        from contextlib import ExitStack
        with ExitStack() as ctx:
            dram = ctx.enter_context(tc.tile_pool(name="dram", bufs=1, space="DRAM"))
            const = ctx.enter_context(tc.tile_pool(name="const", bufs=1))
            big = ctx.enter_context(tc.tile_pool(name="big", bufs=1))
            work = ctx.enter_context(tc.tile_pool(name="work", bufs=3))
            gbuf = ctx.enter_context(tc.tile_pool(name="gbuf", bufs=3))
            stat = ctx.enter_context(tc.tile_pool(name="stat", bufs=4))

            h_own0 = dram.tile([H5 * P, Hdim], f8)
            h_t0 = dram.tile([HALF_N, Hdim], f8, addr_space="Shared")
            # packed second collective: rows 0-639 = h slots 5-9 (fp8),
            # rows 640-1663 = own image encodings [m, colhalf, p] (fp8)
            comb_in = dram.tile([1664, Hdim], f8)
            comb_G = dram.tile([NCORES * 1664, Hdim], f8, addr_space="Shared")

            # ---- constants in SBUF ----
            wg1_s = const.tile([P, 4, Hdim], bf16)
            nc.sync.dma_start(out=wg1_s[:], in_=t_wg1[:])
            wg2_s = const.tile([P, 2, 4, P], bf16)
            nc.sync.dma_start(out=wg2_s[:], in_=t_wg2k[:])
            wi1_s = const.tile([P, 4, 2, P], bf16)
            nc.sync.dma_start(out=wi1_s[:], in_=t_wi1[:])
            wi2_s = const.tile([P, 2, 4, P], bf16)
            nc.sync.dma_start(out=wi2_s[:], in_=t_wi2[:])
            bg1_s = const.tile([1, Hdim], bf16)
            nc.sync.dma_start(out=bg1_s[:], in_=t_bg1[:])
            bg2_s = const.tile([1, D], bf16)
            nc.sync.dma_start(out=bg2_s[:], in_=t_bg2[:])
            bi1_s = const.tile([P, 2], fp32)
            nc.sync.dma_start(out=bi1_s[:], in_=t_bi1[:])
            bi2_s = const.tile([P, 4], fp32)
            nc.sync.dma_start(out=bi2_s[:], in_=t_bi2[:])
            labf_s = const.tile([P, RT], fp32)
            nc.sync.dma_start(out=labf_s[:], in_=t_labf[:])
            gidx1_s = const.tile([P, ST1 * 8], i16)
            nc.sync.dma_start(out=gidx1_s[:], in_=t_gidx1[:])
            gidx2_s = const.tile([P, ST2 * 8], i16)
            nc.sync.dma_start(out=gidx2_s[:], in_=t_gidx2[:])
            imt_s = const.tile([P, 4, 512], bf16)
            nc.sync.dma_start(out=imt_s[:], in_=t_imt[:])
            ones_row = const.tile([1, P], bf16)
            nc.vector.memset(ones_row[:], 1.0)
            ones_cb = const.tile([P, 1], bf16)
            nc.vector.memset(ones_cb[:], 1.0)
            ones_col = const.tile([P, 1], fp32)
            nc.vector.memset(ones_col[:], 1.0)
            ident_b = const.tile([P, P], bf16)
            make_identity(nc, ident_b[:])

            imgown8 = big.tile([P, 4, 512], f8)     # own images encoded (fp8)
            imgownT = big.tile([P, 4, 512], bf16)   # bf16 copy of the same values
            imgT_s = big.tile([P, 4, BATCH], f8)    # full imgT after AllGather
            txtT8 = big.tile([P, RT, 4, P], f8)     # txtT per row tile (fp8)
            txtT_s = big.tile([P, RT, 4, P], bf16)  # bf16 copy of same values (diag)
            diag_s = stat.tile([P, RT], fp32)
            contrib = stat.tile([P, RT], fp32)

            # ===== image MLP on own 512 images (fills L1 warmup) ==============
            with tc.tile_pool(name="ps_mlp", bufs=2, space="PSUM") as ps_mlp:
                h1t = big.tile([P, 2, 512], bf16)
                for m in range(2):
                    pm = ps_mlp.tile([P, 512], fp32, tag="mlp1")
                    for k in range(4):
                        nc.tensor.matmul(
                            out=pm[:], lhsT=wi1_s[:, k, m, :], rhs=imt_s[:, k, :],
                            start=(k == 0), stop=(k == 3),
                        )
                    nc.scalar.activation(
                        out=h1t[:, m, :], in_=pm[:], func=AF.Relu,
                        bias=bi1_s[:, m:m + 1], scale=1.0,
                    )
                for m in range(4):
                    pm2 = ps_mlp.tile([P, 512], fp32, tag="mlp2")
                    for k in range(2):
                        nc.tensor.matmul(
                            out=pm2[:], lhsT=wi2_s[:, k, m, :], rhs=h1t[:, k, :],
                            start=(k == 0), stop=(k == 1),
                        )
                    nc.scalar.activation(
                        out=imgown8[:, m, :], in_=pm2[:], func=AF.Relu,
                        bias=bi2_s[:, m:m + 1], scale=1.0,
                    )
                    for hcol in range(2):
                        nc.sync.dma_start(
                            out=comb_in[640 + m * 256 + hcol * P:
                                        640 + m * 256 + (hcol + 1) * P, :],
                            in_=imgown8[:, m, hcol * 256:(hcol + 1) * 256],
                        )
                nc.vector.tensor_copy(out=imgownT[:], in_=imgown8[:])


            # ===== GCN layer 1: my 10 slots ===================================
            T1h = (max(T1) + 3) // 4
            o1 = [0]
            for t in T1:
                o1.append(o1[-1] + t)

            def l1_slot(s, ps_ag):
                Ts = T1[s]
                j0 = o1[s]
                cm = gbuf.tile([P, max(T1), P], f8, tag="cm", name="cm")
                nc.scalar.dma_start(out=cm[:, :Ts, :], in_=t_cmat1[:, j0:j0 + Ts, :])
                pa = ps_ag.tile([P, D], fp32, tag="agg1", name="pa")
                qs = [(i * Ts) // 4 for i in range(5)]
                parts = [(qs[i], qs[i + 1]) for i in range(4) if qs[i + 1] > qs[i]]
                ghs = []
                for qi, (a, b) in enumerate(parts):
                    gh = gbuf.tile([P, T1h, D], f8, tag=f"g1_{qi}", name="gh")
                    nc.gpsimd.dma_gather(
                        out_ap=gh[:, :b - a, :], in_ap=t_xrow[:, :],
                        idxs_ap=gidx1_s[:, (j0 + a) * 8:(j0 + b) * 8],
                        num_idxs=(b - a) * P, num_idxs_reg=(b - a) * P,
                        elem_size=D, single_packet=False,
                        queue_num=(s + qi) % 4,
                    )
                    ghs.append(gh)
                for qi, (a, b) in enumerate(parts):
                    j = a
                    while j < b:
                        if j + 1 < b:
                            nc.tensor.matmul(
                                out=pa[:], lhsT=cm[:, j:j + 2, :],
                                rhs=ghs[qi][:, j - a:j - a + 2, :],
                                start=(j == 0), stop=(j + 2 == Ts), perf_mode=DR,
                            )
                            j += 2
                        else:
                            nc.tensor.matmul(
                                out=pa[:], lhsT=cm[:, j, :], rhs=ghs[qi][:, j - a, :],
                                start=(j == 0), stop=(j + 1 == Ts),
                            )
                            j += 1
                a1 = work.tile([P, D], bf16, tag="a1", name="a1")
                nc.vector.tensor_copy(out=a1[:], in_=pa[:])
                a1t = work.tile([P, 4, P], bf16, tag="a1t", name="a1t")
                for k in range(4):
                    pt1 = ps_ag.tile([P, P], bf16, tag="tps1", name="pt1")
                    nc.tensor.transpose(
                        out=pt1[:], in_=a1[:, k * P:(k + 1) * P], identity=ident_b[:]
                    )
                    nc.vector.tensor_copy(out=a1t[:, k, :], in_=pt1[:])
                ph = ps_ag.tile([P, Hdim], fp32, tag="hps", name="ph")
                for k in range(4):
                    nc.tensor.matmul(
                        out=ph[:], lhsT=a1t[:, k, :], rhs=wg1_s[:, k, :],
                        start=(k == 0), stop=False,
                    )
                nc.tensor.matmul(
                    out=ph[:], lhsT=ones_row[:], rhs=bg1_s[:],
                    start=False, stop=True, skip_group_check=True,
                )
                h_sb = work.tile([P, Hdim], f8, tag="h_sb", name="h_sb")
                nc.scalar.activation(out=h_sb[:], in_=ph[:], func=AF.Relu)
                if s < H5:
                    nc.sync.dma_start(out=h_own0[s * P:(s + 1) * P, :], in_=h_sb[:])
                else:
                    nc.sync.dma_start(
                        out=comb_in[(s - H5) * P:(s - H5 + 1) * P, :], in_=h_sb[:]
                    )

            with tc.tile_pool(name="ps_ag", bufs=2, space="PSUM") as ps_ag:
                for s in range(H5):
                    l1_slot(s, ps_ag)
                nc.gpsimd.collective_compute(
                    "AllGather", mybir.AluOpType.bypass, replica_groups=rg,
                    ins=[h_own0[:, :]], outs=[h_t0[:, :]],
                )
                for s in range(H5, CPC):
                    l1_slot(s, ps_ag)

            # ===== GCN layer 2, phase A (h_t0 sources; overlaps 2nd AllGather)
            T2m = max(max(T2A), max(T2B))
            a2A_s = big.tile([P, RT, Hdim], bf16)   # partial aggregates (A half)
            c2 = ExitStack()
            ps_l2 = c2.enter_context(tc.tile_pool(name="ps_l2", bufs=1, space="PSUM"))
            ps_tx = ps_l2
            cm2 = big.tile([P, ST2, P], f8)
            nc.scalar.dma_start(out=cm2[:], in_=t_cmat2[:])
            ga_t = []
            for r in range(RT):
                ga = gbuf.tile([P, T2m, Hdim], f8, tag=f"g2a{r}", name="ga", bufs=1)
                nc.gpsimd.dma_gather(
                    out_ap=ga[:, :T2A[r], :], in_ap=h_t0[:, :],
                    idxs_ap=gidx2_s[:, offA[r] * 8:offA[r + 1] * 8],
                    num_idxs=T2A[r] * P, num_idxs_reg=T2A[r] * P,
                    elem_size=Hdim, single_packet=False,
                    queue_num=r % 4,
                )
                ga_t.append(ga)
            for r in range(RT):
                pa2 = ps_l2.tile([P, Hdim], fp32, tag="agg2", name="pa2", bufs=2)
                ga = ga_t[r]
                j = 0
                while j < T2A[r]:
                    if j + 1 < T2A[r]:
                        nc.tensor.matmul(
                            out=pa2[:], lhsT=cm2[:, offA[r] + j:offA[r] + j + 2, :],
                            rhs=ga[:, j:j + 2, :], start=(j == 0),
                            stop=(j + 2 == T2A[r]), perf_mode=DR,
                        )
                        j += 2
                    else:
                        nc.tensor.matmul(
                            out=pa2[:], lhsT=cm2[:, offA[r] + j, :], rhs=ga[:, j, :],
                            start=(j == 0), stop=(j + 1 == T2A[r]),
                        )
                        j += 1
                nc.vector.tensor_copy(out=a2A_s[:, r, :], in_=pa2[:])

            # 2nd AllGather: h slots 5-9 + own image encodings, packed
            nc.gpsimd.collective_compute(
                "AllGather", mybir.AluOpType.bypass, replica_groups=rg,
                ins=[comb_in[:, :]], outs=[comb_G[:, :]],
            )
            # full imgT into SBUF (overlaps phase-B gathers)
            for n in range(NT):
                for k in range(4):
                    for hcol in range(2):
                        nc.sync.dma_start(
                            out=imgT_s[:, k, n * 512 + hcol * 256:
                                       n * 512 + (hcol + 1) * 256],
                            in_=comb_G[n * 1664 + 640 + k * 256 + hcol * P:
                                       n * 1664 + 640 + k * 256 + (hcol + 1) * P, :],
                        )

            # ===== phase B (comb_G sources) + txtT + diag =====================
            gb_t = []
            for r in range(RT):
                gb = gbuf.tile([P, T2m, Hdim], f8, tag=f"g2b{r}", name="gb", bufs=1)
                nc.gpsimd.dma_gather(
                    out_ap=gb[:, :T2B[r], :], in_ap=comb_G[:, :],
                    idxs_ap=gidx2_s[:, offB[r] * 8:offB[r + 1] * 8],
                    num_idxs=T2B[r] * P, num_idxs_reg=T2B[r] * P,
                    elem_size=Hdim, single_packet=False,
                    queue_num=r % 4,
                )
                gb_t.append(gb)
            for r in range(RT):
                pa2 = ps_l2.tile([P, Hdim], fp32, tag="agg2", name="pa2b", bufs=2)
                gb = gb_t[r]
                j = 0
                while j < T2B[r]:
                    if j + 1 < T2B[r]:
                        nc.tensor.matmul(
                            out=pa2[:], lhsT=cm2[:, offB[r] + j:offB[r] + j + 2, :],
                            rhs=gb[:, j:j + 2, :], start=(j == 0),
                            stop=(j + 2 == T2B[r]), perf_mode=DR,
                        )
                        j += 2
                    else:
                        nc.tensor.matmul(
                            out=pa2[:], lhsT=cm2[:, offB[r] + j, :], rhs=gb[:, j, :],
                            start=(j == 0), stop=(j + 1 == T2B[r]),
                        )
                        j += 1
                a2b = work.tile([P, Hdim], bf16, tag="a2b")
                nc.vector.tensor_copy(out=a2b[:], in_=pa2[:])
                a2 = work.tile([P, Hdim], bf16, tag="a2")
                nc.vector.tensor_add(out=a2[:], in0=a2A_s[:, r, :], in1=a2b[:])
                a2t = work.tile([P, 2, P], bf16, tag="a2t")
                for k in range(2):
                    pt = ps_l2.tile([P, P], bf16, tag="tps")
                    nc.tensor.transpose(
                        out=pt[:], in_=a2[:, k * P:(k + 1) * P], identity=ident_b[:]
                    )
                    nc.vector.tensor_copy(out=a2t[:, k, :], in_=pt[:])
                # txtT[d block] = W2[:,d]^T @ agg2^T + b2[d]
                dprod = work.tile([P, 4, P], bf16, tag="dprod")
                for d in range(4):
                    ptx = ps_tx.tile([P, P], fp32, tag="ptx")
                    for k in range(2):
                        nc.tensor.matmul(
                            out=ptx[:], lhsT=wg2_s[:, k, d, :], rhs=a2t[:, k, :],
                            start=(k == 0), stop=False,
                        )
                    nc.tensor.matmul(
                        out=ptx[:], lhsT=bg2_s[:, d * P:(d + 1) * P], rhs=ones_row[:],
                        start=False, stop=True, skip_group_check=True,
                    )
                    nc.vector.tensor_copy(out=txtT8[:, r, d, :], in_=ptx[:])
                    nc.vector.tensor_copy(out=txtT_s[:, r, d, :], in_=txtT8[:, r, d, :])
                    nc.vector.tensor_tensor(
                        out=dprod[:, d, :], in0=txtT_s[:, r, d, :],
                        in1=imgownT[:, d, r * P:(r + 1) * P],
                        op=mybir.AluOpType.mult,
                    )
                pd = ps_tx.tile([P, 1], fp32, tag="pd")
                for d in range(4):
                    nc.tensor.matmul(
                        out=pd[:], lhsT=dprod[:, d, :], rhs=ones_cb[:],
                        start=(d == 0), stop=(d == 3),
                    )
                nc.vector.tensor_copy(out=diag_s[:, r:r + 1], in_=pd[:])
                # ---- logits + row losses for this row tile ----
                sums = stat.tile([P, NT], fp32, tag="sums")
                if stable:
                    banks = []
                for n in range(NT):
                    pl = ps_l2.tile([P, 512], fp32, tag="lg", bufs=2)
                    for g in range(2):
                        nc.tensor.matmul(
                            out=pl[:], lhsT=txtT8[:, r, 2 * g:2 * g + 2, :],
                            rhs=imgT_s[:, 2 * g:2 * g + 2, n * 512:(n + 1) * 512],
                            start=(g == 0), stop=(g == 1), perf_mode=DR,
                        )
                    if stable:
                        banks.append(pl)
                    else:
                        esc = work.tile([P, 512], bf16, tag="esc")
                        nc.scalar.activation(
                            out=esc[:], in_=pl[:], func=AF.Exp,
                            accum_out=sums[:, n:n + 1],
                        )
                if stable:
                    maxes = stat.tile([P, NT], fp32, tag="maxes")
                    for n in range(NT):
                        nc.vector.reduce_max(out=maxes[:, n:n + 1], in_=banks[n][:], axis=AX.X)
                    rmax = stat.tile([P, 1], fp32, tag="rmax")
                    nc.vector.reduce_max(out=rmax[:], in_=maxes[:], axis=AX.X)
                    nrmax = stat.tile([P, 1], fp32, tag="nrmax")
                    nc.scalar.mul(nrmax[:], rmax[:], -1.0)
                    for n in range(NT):
                        esc = work.tile([P, 512], bf16, tag="esc")
                        nc.scalar.activation(
                            out=esc[:], in_=banks[n][:], func=AF.Exp,
                            bias=nrmax[:], scale=1.0, accum_out=sums[:, n:n + 1],
                        )
                ssum = stat.tile([P, 1], fp32, tag="ssum")
                nc.vector.reduce_sum(out=ssum[:], in_=sums[:], axis=AX.X)
                lns = stat.tile([P, 1], fp32, tag="lns")
                nc.scalar.activation(out=lns[:], in_=ssum[:], func=AF.Ln)
                t1 = stat.tile([P, 1], fp32, tag="t1")
                if stable:
                    nc.vector.tensor_add(out=t1[:], in0=rmax[:], in1=lns[:])
                    nc.vector.tensor_sub(out=t1[:], in0=t1[:], in1=diag_s[:, r:r + 1])
                else:
                    nc.vector.tensor_sub(out=t1[:], in0=lns[:], in1=diag_s[:, r:r + 1])
                nc.vector.tensor_mul(
                    out=contrib[:, r:r + 1], in0=t1[:], in1=labf_s[:, r:r + 1]
                )
            c2.close()
            rsum = stat.tile([P, 1], fp32, tag="rsum")
            nc.vector.reduce_sum(out=rsum[:], in_=contrib[:], axis=AX.X)
            with tc.tile_pool(name="ps_fin", bufs=1, space="PSUM") as ps_fin:
                pf = ps_fin.tile([1, 1], fp32)
                nc.tensor.matmul(out=pf[:], lhsT=rsum[:], rhs=ones_col[:], start=True, stop=True)
                fin = stat.tile([1, 1], fp32, tag="fin")
                nc.vector.tensor_copy(out=fin[:], in_=pf[:])
            nc.sync.dma_start(out=t_out[:], in_=fin[:])

    nc.compile()
    return nc


_CACHE = {}


def kernel(**inputs) -> np.ndarray:
    from concourse.bass_utils import run_bass_kernel_spmd

    shared, percore, key = _prep(inputs)
    ckey = (key[0], key[1], key[2], key[3])
    if ckey not in _CACHE:
        _CACHE[ckey] = _build(ckey)
    nc = _CACHE[ckey]

    in_maps = []
    for c in range(NCORES):
        m = dict(shared)
        pc = percore[c]
        m.update({"cmat1": pc["cmat1"], "gidx1": pc["gidx1"],
                  "cmat2": pc["cmat2"], "gidx2": pc["gidx2"],
                  "imt": pc["imt"], "labf": pc["labf"]})
        in_maps.append(m)

    trace = bool(int(os.environ.get("KERNEL_TRACE", "0")))
    try:
        res = run_bass_kernel_spmd(nc, in_maps, core_ids=list(range(NCORES)), trace=trace)
    except Exception:
        # transient NRT/device hiccups have been observed to clear on retry
        res = run_bass_kernel_spmd(nc, in_maps, core_ids=list(range(NCORES)), trace=trace)
    kernel.last_results = res
    total = sum(float(r["partial"][0, 0]) for r in res.results)
    return np.float32(total / BATCH + 1.0)


# revision 24
# speedup vs baseline: 1.1985x; 1.0777x over previous
"""Trainium2 Bass kernel for nn_CLIP_GCN_Model (2-layer GCN + MLP + contrastive loss).

Reformulation (validated numerically):
  out = mean_i(label_i * (lse_i - logits_ii)) + 1.0
(the triplet term of the reference is identically 1.0).

GCN layer: out = S @ (x @ W) + b where S = D^-1/2 (A+I) D^-1/2.
Layer 1 runs over all 10240 (padded) nodes: 80 dst-chunks of 128 nodes,
assigned to (core, slot) with per-slot tile counts T1[s]; per chunk the
distinct source rows are gathered (dedup) and aggregated with a coefficient
matrix C (TensorE matmuls in PSUM), then W_g1 + bias + relu -> h.
h is AllGathered (two halves, Shared-output fast path) into h_t.

Layer 2 only computes the rows actually consumed by the loss: each core owns
batch rows [512c, 512c+512) (4 row-tiles of 128). Per row-tile the distinct
in-edge sources of the rows' label nodes are gathered from h_t (split into
h_t-half-0 / half-1 groups so half-0 aggregation starts right after the first
AllGather) and aggregated directly into [128 rows, 256]; W_g2 is applied via
transposed matmuls producing txtT [512, rows] in SBUF directly.

The image MLP is data-parallel: each core encodes its own 512 images
(transposed layout), AllGathers the result (Shared), and the full [512, 4096]
imgT is used as logits rhs. The diagonal logits are computed locally as
columnwise dots of txtT with the core's own image block. Softmax skips the
row-max subtraction when a host-side bound check shows exp cannot overflow
(max |logit| ~ 10 for this data regime); otherwise a stable variant is built.
"""

import os
import numpy as np
import ml_dtypes

BF16 = ml_dtypes.bfloat16
F8 = ml_dtypes.float8_e4m3   # TRN fp8e4 (max 240)

N_NODES = 10000
NPAD = 10240
D = 512
Hdim = 256
BATCH = 4096
NCORES = 8
P = 128
NCHUNK = NPAD // P          # 80
CPC = NCHUNK // NCORES      # 10 slots per core
NPC = NPAD // NCORES        # 1280 nodes per core
RT = 4                      # row tiles per core (512 rows each core)
NT = BATCH // 512           # 8 column tiles of 512


def _wrap16(idx, n):
    """Layout indices for dma_gather: element i -> [i%16, i//16], replicated to 128 partitions."""
    assert len(idx) == n and n % 16 == 0
    base = idx.astype(np.int16).reshape(n // 16, 16).T  # [16, n/16]
    return np.ascontiguousarray(np.tile(base, (8, 1)))  # [128, n/16]


def _prep(inputs):
    """Host-side layout/sharding prep."""
    x = np.asarray(inputs["x_nodes"], dtype=np.float32)
    image = np.asarray(inputs["image"], dtype=np.float32)
    ei = np.asarray(inputs["edge_index"]).astype(np.int64)
    label = np.asarray(inputs["label"]).astype(np.int64)
    src, dst = ei[0], ei[1]

    deg = np.ones(N_NODES, np.float32)
    np.add.at(deg, dst, 1.0)
    dinv = (1.0 / np.sqrt(deg)).astype(np.float32)

    # in-edges grouped by dst (sorted once)
    order = np.argsort(dst, kind="stable")
    src_s, dst_s = src[order], dst[order]
    bound = np.searchsorted(dst_s, np.arange(N_NODES + 1))

    # ---------------- L1: per-chunk dedup + balanced (core,slot) assignment --
    chunk_src = []      # distinct sources per chunk
    chunk_C = []        # [n_distinct, 128] fp32 coef
    for c in range(NCHUNK):
        n0, n1 = c * P, min((c + 1) * P, N_NODES)
        if n0 >= N_NODES:
            chunk_src.append(np.zeros(1, np.int64))
            chunk_C.append(np.zeros((1, P), np.float32))
            continue
        e0, e1 = bound[n0], bound[n1]
        es, ed = src_s[e0:e1], dst_s[e0:e1]
        selfn = np.arange(n0, n1)
        all_s = np.concatenate([es, selfn])
        all_d = np.concatenate([ed, selfn]) - n0
        coef = np.concatenate([dinv[es] * dinv[ed], dinv[selfn] ** 2])
        uniq, inv = np.unique(all_s, return_inverse=True)
        C = np.zeros((len(uniq), P), np.float32)
        np.add.at(C, (inv, all_d), coef)
        chunk_src.append(uniq)
        chunk_C.append(C)

    counts = np.array([len(s) for s in chunk_src])
    rank = np.argsort(-counts, kind="stable")
    a_k = np.zeros(NCHUNK, np.int64)   # chunk -> core
    s_k = np.zeros(NCHUNK, np.int64)   # chunk -> slot
    T1 = []
    for s in range(CPC):
        grp = rank[s * NCORES:(s + 1) * NCORES]
        a_k[grp] = np.arange(NCORES)
        s_k[grp] = s
        T1.append(int(np.ceil(counts[grp].max() / P)))
    T1 = tuple(T1)
    ST1 = sum(T1)
    off1 = np.concatenate([[0], np.cumsum(T1)])

    # node -> gather-table row: slots 0-4 land in h_t0 (AllGather 1, rows
    # core*640 + slot*128 + p); slots 5-9 land in the packed comb_G (AllGather 2,
    # rows core*1664 + (slot-5)*128 + p), keyed with a +5120 offset.
    kk = np.arange(NPAD) // P
    pp_ = np.arange(NPAD) % P
    hrow = np.where(
        s_k[kk] < CPC // 2,
        a_k[kk] * (NPC // 2) + s_k[kk] * P + pp_,
        NPAD // 2 + a_k[kk] * 1664 + (s_k[kk] - CPC // 2) * P + pp_,
    )

    gidx1 = np.zeros((NCORES, P, ST1 * 8), np.int16)
    cmat1 = np.zeros((NCORES, P, ST1, P), F8)
    for c in range(NCHUNK):
        cr, sl = a_k[c], s_k[c]
        E_s = T1[sl] * P
        idxs = np.zeros(E_s, np.int64)
        idxs[:counts[c]] = chunk_src[c]
        gidx1[cr, :, off1[sl] * 8:off1[sl + 1] * 8] = _wrap16(idxs, E_s)
        Cp = np.zeros((E_s, P), np.float32)
        Cp[:counts[c]] = chunk_C[c]
        # edge-slot e -> [partition e%128, tile e//128]
        cmat1[cr, :, off1[sl]:off1[sl + 1], :] = \
            Cp.reshape(T1[sl], P, P).transpose(1, 0, 2).astype(F8)

    # ---------------- L2: per-row-tile (labeled dst only), h_t-half split ----
    HALF_N = NPAD // 2
    bins = label.reshape(NCORES, RT, P)   # core c, tile r, row p -> label node
    t2a = np.zeros((NCORES, RT), np.int64)
    t2b = np.zeros((NCORES, RT), np.int64)
    binsrc = {}
    for c in range(NCORES):
        for r in range(RT):
            labs = bins[c, r]
            segs, segd, segc = [], [], []
            for p in range(P):
                v = labs[p]
                e0, e1 = bound[v], bound[v + 1]
                es = src_s[e0:e1]
                segs.append(np.concatenate([es, [v]]))
                segd.append(np.full(len(es) + 1, p, np.int64))
                segc.append(np.concatenate([dinv[es] * dinv[v], [dinv[v] ** 2]]))
            all_s = np.concatenate(segs)
            all_d = np.concatenate(segd)
            coef = np.concatenate(segc)
            hr = hrow[all_s]
            uniq, inv = np.unique(hr, return_inverse=True)
            C = np.zeros((len(uniq), P), np.float32)
            np.add.at(C, (inv, all_d), coef)
            na = int((uniq < HALF_N).sum())   # uniq sorted -> half0 first
            t2a[c, r] = int(np.ceil(max(na, 1) / P))
            t2b[c, r] = int(np.ceil(max(len(uniq) - na, 1) / P))
            binsrc[(c, r)] = (uniq, C, na)
    T2A = tuple(int(t2a[:, r].max()) for r in range(RT))
    T2B = tuple(int(t2b[:, r].max()) for r in range(RT))
    ST2 = sum(T2A) + sum(T2B)
    offA = np.concatenate([[0], np.cumsum(T2A)])
    base_b = offA[-1]
    offB = base_b + np.concatenate([[0], np.cumsum(T2B)])

    gidx2 = np.zeros((NCORES, P, ST2 * 8), np.int16)
    cmat2 = np.zeros((NCORES, P, ST2, P), F8)
    for c in range(NCORES):
        for r in range(RT):
            uniq, C, na = binsrc[(c, r)]
            nb = len(uniq) - na
            Ea, Eb = T2A[r] * P, T2B[r] * P
            ia = np.zeros(Ea, np.int64)
            ia[:na] = uniq[:na]                      # rows into h_t[0:5120]
            ib = np.zeros(Eb, np.int64)
            ib[:nb] = uniq[na:] - HALF_N             # rows into h_t[5120:10240]
            gidx2[c, :, offA[r] * 8:offA[r + 1] * 8] = _wrap16(ia, Ea)
            gidx2[c, :, offB[r] * 8:offB[r + 1] * 8] = _wrap16(ib, Eb)
            Ca = np.zeros((Ea, P), np.float32)
            Ca[:na] = C[:na]
            Cb = np.zeros((Eb, P), np.float32)
            Cb[:nb] = C[na:]
            cmat2[c, :, offA[r]:offA[r + 1], :] = \
                Ca.reshape(T2A[r], P, P).transpose(1, 0, 2).astype(F8)
            cmat2[c, :, offB[r]:offB[r + 1], :] = \
                Cb.reshape(T2B[r], P, P).transpose(1, 0, 2).astype(F8)

    # ---------------- softmax-stability bound (cheap fp32 host forward) ------
    def _agg_all(xw):
        # fast segment sum via reduceat on the dst-sorted edges
        msg = (dinv[src_s] * dinv[dst_s])[:, None] * xw[src_s]
        agg = np.zeros_like(xw)
        nz = np.flatnonzero(np.diff(np.append(-1, dst_s)))
        agg[dst_s[nz]] = np.add.reduceat(msg, nz, axis=0)
        return agg + (dinv * dinv)[:, None] * xw

    h_np = np.maximum(_agg_all(x @ np.asarray(inputs["W_g1"], np.float32))
                      + np.asarray(inputs["b_g1"], np.float32), 0.0)
    g_np = _agg_all(h_np @ np.asarray(inputs["W_g2"], np.float32)) \
        + np.asarray(inputs["b_g2"], np.float32)
    img_np = np.maximum(image @ np.asarray(inputs["W_img1"], np.float32)
                        + np.asarray(inputs["b_img1"], np.float32), 0.0)
    img_np = np.maximum(img_np @ np.asarray(inputs["W_img2"], np.float32)
                        + np.asarray(inputs["b_img2"], np.float32), 0.0)
    bnd_logit = float(np.linalg.norm(g_np[label], axis=1).max()
                      * np.linalg.norm(img_np, axis=1).max())
    stable = bnd_logit > 60.0

    # ---------------- tensors ------------------------------------------------
    xpad = np.zeros((NPAD, D), np.float32)
    xpad[:N_NODES] = x
    xrow = np.ascontiguousarray(xpad).astype(F8)

    def km(w, kt):  # [K, M] -> [128p, kt, M]
        return np.ascontiguousarray(
            w.reshape(kt, P, w.shape[1]).transpose(1, 0, 2)
        ).astype(BF16)

    shared = {
        "xrow": xrow,
        "wg1": km(np.asarray(inputs["W_g1"], np.float32), 4),       # [128, 4, 256]
        "wg2k": np.ascontiguousarray(
            np.asarray(inputs["W_g2"], np.float32).reshape(2, P, 4, P).transpose(1, 0, 2, 3)
        ).astype(BF16),                                             # [128, 2k, 4d, 128]
        "wi1": np.ascontiguousarray(
            np.asarray(inputs["W_img1"], np.float32).reshape(4, P, 2, P).transpose(1, 0, 2, 3)
        ).astype(BF16),
        "wi2": np.ascontiguousarray(
            np.asarray(inputs["W_img2"], np.float32).reshape(2, P, 4, P).transpose(1, 0, 2, 3)
        ).astype(BF16),
        "bg1": np.asarray(inputs["b_g1"], np.float32).astype(BF16).reshape(1, Hdim),
        "bg2": np.asarray(inputs["b_g2"], np.float32).astype(BF16).reshape(1, D),
        "bi1": np.ascontiguousarray(np.asarray(inputs["b_img1"], np.float32).reshape(2, P).T),
        "bi2": np.ascontiguousarray(np.asarray(inputs["b_img2"], np.float32).reshape(4, P).T),
    }

    imageb = image.astype(BF16)
    percore = []
    for c in range(NCORES):
        imt = np.ascontiguousarray(
            imageb[c * 512:(c + 1) * 512].T.reshape(4, P, 512).transpose(1, 0, 2)
        )  # [128 kpart, 4 kblk, 512 own imgs]
        labf = np.ascontiguousarray(
            label[c * 512:(c + 1) * 512].astype(np.float32).reshape(RT, P).T
        )  # [128, RT]
        percore.append({
            "cmat1": np.ascontiguousarray(cmat1[c]),
            "gidx1": np.ascontiguousarray(gidx1[c]),
            "cmat2": np.ascontiguousarray(cmat2[c]),
            "gidx2": np.ascontiguousarray(gidx2[c]),
            "imt": imt, "labf": labf,
        })
    return shared, percore, (T1, T2A, T2B, stable)


def _build(key):
    """Build the SPMD Bass program."""
    T1, T2A, T2B, stable = key
    import concourse.bass as bass  # noqa: F401
    import concourse.tile as tile
    from concourse import bacc, mybir
    from concourse.masks import make_identity

    fp32 = mybir.dt.float32
    bf16 = mybir.dt.bfloat16
    f8 = mybir.dt.float8e4
    i16 = mybir.dt.int16
    AF = mybir.ActivationFunctionType
    DR = mybir.MatmulPerfMode.DoubleRow
    AX = mybir.AxisListType
    ST1 = sum(T1)
    ST2 = sum(T2A) + sum(T2B)
    offA = [0]
    for t in T2A:
        offA.append(offA[-1] + t)
    offB = [offA[-1]]
    for t in T2B:
        offB.append(offB[-1] + t)
    H5 = CPC // 2
    HALF_N = NPAD // 2

    nc = bacc.Bacc("TRN2", target_bir_lowering=False, debug=False,
                   num_devices=NCORES, num_swdge_queues=4)

    t_xrow = nc.dram_tensor("xrow", [NPAD, D], f8, kind="ExternalInput").ap()
    t_wg1 = nc.dram_tensor("wg1", [P, 4, Hdim], bf16, kind="ExternalInput").ap()
    t_wg2k = nc.dram_tensor("wg2k", [P, 2, 4, P], bf16, kind="ExternalInput").ap()
    t_wi1 = nc.dram_tensor("wi1", [P, 4, 2, P], bf16, kind="ExternalInput").ap()
    t_wi2 = nc.dram_tensor("wi2", [P, 2, 4, P], bf16, kind="ExternalInput").ap()
    t_bg1 = nc.dram_tensor("bg1", [1, Hdim], bf16, kind="ExternalInput").ap()
    t_bg2 = nc.dram_tensor("bg2", [1, D], bf16, kind="ExternalInput").ap()
    t_bi1 = nc.dram_tensor("bi1", [P, 2], fp32, kind="ExternalInput").ap()
    t_bi2 = nc.dram_tensor("bi2", [P, 4], fp32, kind="ExternalInput").ap()
    t_cmat1 = nc.dram_tensor("cmat1", [P, ST1, P], f8, kind="ExternalInput").ap()
    t_gidx1 = nc.dram_tensor("gidx1", [P, ST1 * 8], i16, kind="ExternalInput").ap()
    t_cmat2 = nc.dram_tensor("cmat2", [P, ST2, P], f8, kind="ExternalInput").ap()
    t_gidx2 = nc.dram_tensor("gidx2", [P, ST2 * 8], i16, kind="ExternalInput").ap()
    t_imt = nc.dram_tensor("imt", [P, 4, 512], bf16, kind="ExternalInput").ap()
    t_labf = nc.dram_tensor("labf", [P, RT], fp32, kind="ExternalInput").ap()
    t_out = nc.dram_tensor("partial", [1, 1], fp32, kind="ExternalOutput").ap()

    rg = [list(range(NCORES))]

    with tile.TileContext(nc) as tc:
        from contextlib import ExitStack
        with ExitStack() as ctx:
            dram = ctx.enter_context(tc.tile_pool(name="dram", bufs=1, space="DRAM"))
            const = ctx.enter_context(tc.tile_pool(name="const", bufs=1))
            big = ctx.enter_context(tc.tile_pool(name="big", bufs=1))
            work = ctx.enter_context(tc.tile_pool(name="work", bufs=3))
            gbuf = ctx.enter_context(tc.tile_pool(name="gbuf", bufs=3))
            stat = ctx.enter_context(tc.tile_pool(name="stat", bufs=4))

            h_own0 = dram.tile([H5 * P, Hdim], f8)
            h_t0 = dram.tile([HALF_N, Hdim], f8, addr_space="Shared")
            # packed second collective: rows 0-639 = h slots 5-9 (fp8),
            # rows 640-1663 = own image encodings [m, colhalf, p] (fp8)
            comb_in = dram.tile([1664, Hdim], f8)
            comb_G = dram.tile([NCORES * 1664, Hdim], f8, addr_space="Shared")

            # ---- constants in SBUF ----
            wg1_s = const.tile([P, 4, Hdim], bf16)
            nc.sync.dma_start(out=wg1_s[:], in_=t_wg1[:])
            wg2_s = const.tile([P, 2, 4, P], bf16)
            nc.sync.dma_start(out=wg2_s[:], in_=t_wg2k[:])
            wi1_s = const.tile([P, 4, 2, P], bf16)
            nc.sync.dma_start(out=wi1_s[:], in_=t_wi1[:])
            wi2_s = const.tile([P, 2, 4, P], bf16)
            nc.sync.dma_start(out=wi2_s[:], in_=t_wi2[:])
            bg1_s = const.tile([1, Hdim], bf16)
            nc.sync.dma_start(out=bg1_s[:], in_=t_bg1[:])
            bg2_s = const.tile([1, D], bf16)
            nc.sync.dma_start(out=bg2_s[:], in_=t_bg2[:])
            bi1_s = const.tile([P, 2], fp32)
            nc.sync.dma_start(out=bi1_s[:], in_=t_bi1[:])
            bi2_s = const.tile([P, 4], fp32)
            nc.sync.dma_start(out=bi2_s[:], in_=t_bi2[:])
            labf_s = const.tile([P, RT], fp32)
            nc.sync.dma_start(out=labf_s[:], in_=t_labf[:])
            gidx1_s = const.tile([P, ST1 * 8], i16)
            nc.sync.dma_start(out=gidx1_s[:], in_=t_gidx1[:])
            gidx2_s = const.tile([P, ST2 * 8], i16)
            nc.sync.dma_start(out=gidx2_s[:], in_=t_gidx2[:])
            imt_s = const.tile([P, 4, 512], bf16)
            nc.sync.dma_start(out=imt_s[:], in_=t_imt[:])
            ones_row = const.tile([1, P], bf16)
            nc.vector.memset(ones_row[:], 1.0)
            ones_cb = const.tile([P, 1], bf16)
            nc.vector.memset(ones_cb[:], 1.0)
            ones_col = const.tile([P, 1], fp32)
            nc.vector.memset(ones_col[:], 1.0)
            ident_b = const.tile([P, P], bf16)
            make_identity(nc, ident_b[:])

            imgown8 = big.tile([P, 4, 512], f8)     # own images encoded (fp8)
            imgownT = big.tile([P, 4, 512], bf16)   # bf16 copy of the same values
            imgT_s = big.tile([P, 4, BATCH], f8)    # full imgT after AllGather
            txtT8 = big.tile([P, RT, 4, P], f8)     # txtT per row tile (fp8)
            txtT_s = big.tile([P, RT, 4, P], bf16)  # bf16 copy of same values (diag)
            diag_s = stat.tile([P, RT], fp32)
            contrib = stat.tile([P, RT], fp32)

            # ===== image MLP on own 512 images (fills L1 warmup) ==============
            with tc.tile_pool(name="ps_mlp", bufs=2, space="PSUM") as ps_mlp:
                h1t = big.tile([P, 2, 512], bf16)
                for m in range(2):
                    pm = ps_mlp.tile([P, 512], fp32, tag="mlp1")
                    for k in range(4):
                        nc.tensor.matmul(
                            out=pm[:], lhsT=wi1_s[:, k, m, :], rhs=imt_s[:, k, :],
                            start=(k == 0), stop=(k == 3),
                        )
                    nc.scalar.activation(
                        out=h1t[:, m, :], in_=pm[:], func=AF.Relu,
                        bias=bi1_s[:, m:m + 1], scale=1.0,
                    )
                for m in range(4):
                    pm2 = ps_mlp.tile([P, 512], fp32, tag="mlp2")
                    for k in range(2):
                        nc.tensor.matmul(
                            out=pm2[:], lhsT=wi2_s[:, k, m, :], rhs=h1t[:, k, :],
                            start=(k == 0), stop=(k == 1),
                        )
                    nc.scalar.activation(
                        out=imgown8[:, m, :], in_=pm2[:], func=AF.Relu,
                        bias=bi2_s[:, m:m + 1], scale=1.0,
                    )
                    for hcol in range(2):
                        nc.sync.dma_start(
                            out=comb_in[640 + m * 256 + hcol * P:
                                        640 + m * 256 + (hcol + 1) * P, :],
                            in_=imgown8[:, m, hcol * 256:(hcol + 1) * 256],
                        )
                nc.vector.tensor_copy(out=imgownT[:], in_=imgown8[:])


            # ===== GCN layer 1: my 10 slots ===================================
            T1h = (max(T1) + 3) // 4
            o1 = [0]
            for t in T1:
                o1.append(o1[-1] + t)

            def l1_slot(s, ps_ag):
                Ts = T1[s]
                j0 = o1[s]
                cm = gbuf.tile([P, max(T1), P], f8, tag="cm", name="cm")
                nc.scalar.dma_start(out=cm[:, :Ts, :], in_=t_cmat1[:, j0:j0 + Ts, :])
                pa = ps_ag.tile([P, D], fp32, tag="agg1", name="pa")
                qs = [(i * Ts) // 4 for i in range(5)]
                parts = [(qs[i], qs[i + 1]) for i in range(4) if qs[i + 1] > qs[i]]
                ghs = []
                for qi, (a, b) in enumerate(parts):
                    gh = gbuf.tile([P, T1h, D], f8, tag=f"g1_{qi}", name="gh")
                    nc.gpsimd.dma_gather(
                        out_ap=gh[:, :b - a, :], in_ap=t_xrow[:, :],
                        idxs_ap=gidx1_s[:, (j0 + a) * 8:(j0 + b) * 8],
                        num_idxs=(b - a) * P, num_idxs_reg=(b - a) * P,
                        elem_size=D, single_packet=False,
                        queue_num=(s + qi) % 4,
                    )
                    ghs.append(gh)
                for qi, (a, b) in enumerate(parts):
                    j = a
                    while j < b:
                        if j + 1 < b:
                            nc.tensor.matmul(
                                out=pa[:], lhsT=cm[:, j:j + 2, :],
                                rhs=ghs[qi][:, j - a:j - a + 2, :],
                                start=(j == 0), stop=(j + 2 == Ts), perf_mode=DR,
                            )
                            j += 2
                        else:
                            nc.tensor.matmul(
                                out=pa[:], lhsT=cm[:, j, :], rhs=ghs[qi][:, j - a, :],
                                start=(j == 0), stop=(j + 1 == Ts),
                            )
                            j += 1
                a1 = work.tile([P, D], bf16, tag="a1", name="a1")
                nc.vector.tensor_copy(out=a1[:], in_=pa[:])
                a1t = work.tile([P, 4, P], bf16, tag="a1t", name="a1t")
                for k in range(4):
                    pt1 = ps_ag.tile([P, P], bf16, tag="tps1", name="pt1")
                    nc.tensor.transpose(
                        out=pt1[:], in_=a1[:, k * P:(k + 1) * P], identity=ident_b[:]
                    )
                    nc.vector.tensor_copy(out=a1t[:, k, :], in_=pt1[:])
                ph = ps_ag.tile([P, Hdim], fp32, tag="hps", name="ph")
                for k in range(4):
                    nc.tensor.matmul(
                        out=ph[:], lhsT=a1t[:, k, :], rhs=wg1_s[:, k, :],
                        start=(k == 0), stop=False,
                    )
                nc.tensor.matmul(
                    out=ph[:], lhsT=ones_row[:], rhs=bg1_s[:],
                    start=False, stop=True, skip_group_check=True,
                )
                h_sb = work.tile([P, Hdim], f8, tag="h_sb", name="h_sb")
                nc.scalar.activation(out=h_sb[:], in_=ph[:], func=AF.Relu)
                if s < H5:
                    nc.sync.dma_start(out=h_own0[s * P:(s + 1) * P, :], in_=h_sb[:])
                else:
                    nc.sync.dma_start(
                        out=comb_in[(s - H5) * P:(s - H5 + 1) * P, :], in_=h_sb[:]
                    )

            with tc.tile_pool(name="ps_ag", bufs=2, space="PSUM") as ps_ag:
                for s in range(H5):
                    l1_slot(s, ps_ag)
                nc.gpsimd.collective_compute(
                    "AllGather", mybir.AluOpType.bypass, replica_groups=rg,
                    ins=[h_own0[:, :]], outs=[h_t0[:, :]],
                )
                for s in range(H5, CPC):
                    l1_slot(s, ps_ag)

            # ===== GCN layer 2, phase A (h_t0 sources; overlaps 2nd AllGather)
            T2m = max(max(T2A), max(T2B))
            a2A_s = big.tile([P, RT, Hdim], bf16)   # partial aggregates (A half)
            c2 = ExitStack()
            ps_l2 = c2.enter_context(tc.tile_pool(name="ps_l2", bufs=1, space="PSUM"))
            ps_tx = ps_l2
            cm2 = big.tile([P, ST2, P], f8)
            nc.scalar.dma_start(out=cm2[:], in_=t_cmat2[:])
            ga_t = []
            for r in range(RT):
                ga = gbuf.tile([P, T2m, Hdim], f8, tag=f"g2a{r}", name="ga", bufs=1)
                nc.gpsimd.dma_gather(
                    out_ap=ga[:, :T2A[r], :], in_ap=h_t0[:, :],
                    idxs_ap=gidx2_s[:, offA[r] * 8:offA[r + 1] * 8],
                    num_idxs=T2A[r] * P, num_idxs_reg=T2A[r] * P,
                    elem_size=Hdim, single_packet=False,
                    queue_num=(2 * r) % 4,
                )
                ga_t.append(ga)
            for r in range(RT):
                pa2 = ps_l2.tile([P, Hdim], fp32, tag="agg2", name="pa2", bufs=2)
                ga = ga_t[r]
                j = 0
                while j < T2A[r]:
                    if j + 1 < T2A[r]:
                        nc.tensor.matmul(
                            out=pa2[:], lhsT=cm2[:, offA[r] + j:offA[r] + j + 2, :],
                            rhs=ga[:, j:j + 2, :], start=(j == 0),
                            stop=(j + 2 == T2A[r]), perf_mode=DR,
                        )
                        j += 2
                    else:
                        nc.tensor.matmul(
                            out=pa2[:], lhsT=cm2[:, offA[r] + j, :], rhs=ga[:, j, :],
                            start=(j == 0), stop=(j + 1 == T2A[r]),
                        )
                        j += 1
                nc.vector.tensor_copy(out=a2A_s[:, r, :], in_=pa2[:])

            # 2nd AllGather: h slots 5-9 + own image encodings, packed
            nc.gpsimd.collective_compute(
                "AllGather", mybir.AluOpType.bypass, replica_groups=rg,
                ins=[comb_in[:, :]], outs=[comb_G[:, :]],
            )
            # full imgT into SBUF (overlaps phase-B gathers)
            for n in range(NT):
                for k in range(4):
                    for hcol in range(2):
                        nc.sync.dma_start(
                            out=imgT_s[:, k, n * 512 + hcol * 256:
                                       n * 512 + (hcol + 1) * 256],
                            in_=comb_G[n * 1664 + 640 + k * 256 + hcol * P:
                                       n * 1664 + 640 + k * 256 + (hcol + 1) * P, :],
                        )

            # ===== phase B (comb_G sources) + txtT + diag =====================
            gb_t = []
            for r in range(RT):
                gb = gbuf.tile([P, T2m, Hdim], f8, tag=f"g2b{r}", name="gb", bufs=1)
                nc.gpsimd.dma_gather(
                    out_ap=gb[:, :T2B[r], :], in_ap=comb_G[:, :],
                    idxs_ap=gidx2_s[:, offB[r] * 8:offB[r + 1] * 8],
                    num_idxs=T2B[r] * P, num_idxs_reg=T2B[r] * P,
                    elem_size=Hdim, single_packet=False,
                    queue_num=(2 * r + 1) % 4,
                )
                gb_t.append(gb)
            for r in range(RT):
                pa2 = ps_l2.tile([P, Hdim], fp32, tag="agg2", name="pa2b", bufs=2)
                gb = gb_t[r]
                j = 0
                while j < T2B[r]:
                    if j + 1 < T2B[r]:
                        nc.tensor.matmul(
                            out=pa2[:], lhsT=cm2[:, offB[r] + j:offB[r] + j + 2, :],
                            rhs=gb[:, j:j + 2, :], start=(j == 0),
                            stop=(j + 2 == T2B[r]), perf_mode=DR,
                        )
                        j += 2
                    else:
                        nc.tensor.matmul(
                            out=pa2[:], lhsT=cm2[:, offB[r] + j, :], rhs=gb[:, j, :],
                            start=(j == 0), stop=(j + 1 == T2B[r]),
                        )
                        j += 1
                a2b = work.tile([P, Hdim], bf16, tag="a2b")
                nc.vector.tensor_copy(out=a2b[:], in_=pa2[:])
                a2 = work.tile([P, Hdim], bf16, tag="a2")
                nc.vector.tensor_add(out=a2[:], in0=a2A_s[:, r, :], in1=a2b[:])
                a2t = work.tile([P, 2, P], bf16, tag="a2t")
                for k in range(2):
                    pt = ps_l2.tile([P, P], bf16, tag="tps")
                    nc.tensor.transpose(
                        out=pt[:], in_=a2[:, k * P:(k + 1) * P], identity=ident_b[:]
                    )
                    nc.vector.tensor_copy(out=a2t[:, k, :], in_=pt[:])
                # txtT[d block] = W2[:,d]^T @ agg2^T + b2[d]
                dprod = work.tile([P, 4, P], bf16, tag="dprod")
                for d in range(4):
                    ptx = ps_tx.tile([P, P], fp32, tag="ptx")
                    for k in range(2):
                        nc.tensor.matmul(
                            out=ptx[:], lhsT=wg2_s[:, k, d, :], rhs=a2t[:, k, :],
                            start=(k == 0), stop=False,
                        )
                    nc.tensor.matmul(
                        out=ptx[:], lhsT=bg2_s[:, d * P:(d + 1) * P], rhs=ones_row[:],
                        start=False, stop=True, skip_group_check=True,
                    )
                    nc.vector.tensor_copy(out=txtT8[:, r, d, :], in_=ptx[:])
                    nc.vector.tensor_copy(out=txtT_s[:, r, d, :], in_=txtT8[:, r, d, :])
                    nc.vector.tensor_tensor(
                        out=dprod[:, d, :], in0=txtT_s[:, r, d, :],
                        in1=imgownT[:, d, r * P:(r + 1) * P],
                        op=mybir.AluOpType.mult,
                    )
                pd = ps_tx.tile([P, 1], fp32, tag="pd")
                for d in range(4):
                    nc.tensor.matmul(
                        out=pd[:], lhsT=dprod[:, d, :], rhs=ones_cb[:],
                        start=(d == 0), stop=(d == 3),
                    )
                nc.vector.tensor_copy(out=diag_s[:, r:r + 1], in_=pd[:])
                # ---- logits + row losses for this row tile ----
                sums = stat.tile([P, NT], fp32, tag="sums")
                if stable:
                    banks = []
                for n in range(NT):
                    pl = ps_l2.tile([P, 512], fp32, tag="lg", bufs=2)
                    for g in range(2):
                        nc.tensor.matmul(
                            out=pl[:], lhsT=txtT8[:, r, 2 * g:2 * g + 2, :],
                            rhs=imgT_s[:, 2 * g:2 * g + 2, n * 512:(n + 1) * 512],
                            start=(g == 0), stop=(g == 1), perf_mode=DR,
                        )
                    if stable:
                        banks.append(pl)
                    else:
                        esc = work.tile([P, 512], bf16, tag="esc")
                        nc.scalar.activation(
                            out=esc[:], in_=pl[:], func=AF.Exp,
                            accum_out=sums[:, n:n + 1],
                        )
                if stable:
                    maxes = stat.tile([P, NT], fp32, tag="maxes")
                    for n in range(NT):
                        nc.vector.reduce_max(out=maxes[:, n:n + 1], in_=banks[n][:], axis=AX.X)
                    rmax = stat.tile([P, 1], fp32, tag="rmax")
                    nc.vector.reduce_max(out=rmax[:], in_=maxes[:], axis=AX.X)
                    nrmax = stat.tile([P, 1], fp32, tag="nrmax")
                    nc.scalar.mul(nrmax[:], rmax[:], -1.0)
                    for n in range(NT):
                        esc = work.tile([P, 512], bf16, tag="esc")
                        nc.scalar.activation(
                            out=esc[:], in_=banks[n][:], func=AF.Exp,
                            bias=nrmax[:], scale=1.0, accum_out=sums[:, n:n + 1],
                        )
                ssum = stat.tile([P, 1], fp32, tag="ssum")
                nc.vector.reduce_sum(out=ssum[:], in_=sums[:], axis=AX.X)
                lns = stat.tile([P, 1], fp32, tag="lns")
                nc.scalar.activation(out=lns[:], in_=ssum[:], func=AF.Ln)
                t1 = stat.tile([P, 1], fp32, tag="t1")
                if stable:
                    nc.vector.tensor_add(out=t1[:], in0=rmax[:], in1=lns[:])
                    nc.vector.tensor_sub(out=t1[:], in0=t1[:], in1=diag_s[:, r:r + 1])
                else:
                    nc.vector.tensor_sub(out=t1[:], in0=lns[:], in1=diag_s[:, r:r + 1])
                nc.vector.tensor_mul(
                    out=contrib[:, r:r + 1], in0=t1[:], in1=labf_s[:, r:r + 1]
                )
            c2.close()
            rsum = stat.tile([P, 1], fp32, tag="rsum")
            nc.vector.reduce_sum(out=rsum[:], in_=contrib[:], axis=AX.X)
            with tc.tile_pool(name="ps_fin", bufs=1, space="PSUM") as ps_fin:
                pf = ps_fin.tile([1, 1], fp32)
                nc.tensor.matmul(out=pf[:], lhsT=rsum[:], rhs=ones_col[:], start=True, stop=True)
                fin = stat.tile([1, 1], fp32, tag="fin")
                nc.vector.tensor_copy(out=fin[:], in_=pf[:])
            nc.sync.dma_start(out=t_out[:], in_=fin[:])

    nc.compile()
    return nc


_CACHE = {}


def kernel(**inputs) -> np.ndarray:
    from concourse.bass_utils import run_bass_kernel_spmd

    shared, percore, key = _prep(inputs)
    ckey = (key[0], key[1], key[2], key[3])
    if ckey not in _CACHE:
        _CACHE[ckey] = _build(ckey)
    nc = _CACHE[ckey]

    in_maps = []
    for c in range(NCORES):
        m = dict(shared)
        pc = percore[c]
        m.update({"cmat1": pc["cmat1"], "gidx1": pc["gidx1"],
                  "cmat2": pc["cmat2"], "gidx2": pc["gidx2"],
                  "imt": pc["imt"], "labf": pc["labf"]})
        in_maps.append(m)

    trace = bool(int(os.environ.get("KERNEL_TRACE", "0")))
    try:
        res = run_bass_kernel_spmd(nc, in_maps, core_ids=list(range(NCORES)), trace=trace)
    except Exception:
        # transient NRT/device hiccups have been observed to clear on retry
        res = run_bass_kernel_spmd(nc, in_maps, core_ids=list(range(NCORES)), trace=trace)
    kernel.last_results = res
    total = sum(float(r["partial"][0, 0]) for r in res.results)
    return np.float32(total / BATCH + 1.0)
